# revision 47
# baseline (speedup 1.0000x reference)
"""EventVolumeSurface trilinear voxel-grid kernel for Trainium2 (Bass/Tile).

v10 strategy (data-parallel over batch, 1 batch -> 1 NeuronCore):
  Events are bucketed by (time-segment s in [0,9), y-window q32 = iy>>5 in
  [0,15), x-window r32 = ix>>5 in [0,20)) with straddle duplication at the
  32-boundaries.  Tiles of 128 event slots stream in bucket order, batched
  into groups of G=63.

  Host ships, per 128-event tile slot:
    - rhs [128, T*64] fp8 e3m4: 64 interleaved columns (2*cx + b) holding
      sign * 8 * p * w_b * hx[cx]; hx = 2-tap x hat, w_0 = 1-f, w_1 = f
      (time-bin weights); x8 keeps values in e3m4's normal range (host
      divides by 8 at unshard); sign matches the group route below.
    - scat groups (75%): per-slot y taps (1-fy, fy) f16 + int16
      group-local scatter indices (-1 = skip, handles window straddle).
    - hostm groups (25%): the +hat lhsT tile prebuilt on the host (f16).
    - chain groups (0% by default, kept as a tuning fallback): yhat f32.

  Device m-tile build (columns cost engine time; partitions are free):
    scat:  ONE gpsimd local_scatter builds the whole [128, 63*32] +hat
           lhsT group on Pool (~47ns/tile, replaces sub/abs/min chains)
    hostm: one DMA, zero engine work (spends spare DMA bandwidth to
           relieve Pool; the 75/25 split equalizes Pool vs DMA)
    chain: DVE broadcast-subtract -> ACT Abs -> DVE fused min (= -hat)
    MM: psum[32*g:+32, 64*r:+64] += m_j^T @ rhs_j  (f16 x fp8e3, 64 out
        cols ~36ns; tile_position puts the 32 out rows at partition 32*g)
  PSUM is one [128, 1280] f32 tile per (s, q128) stripe with interleaved
  (x, bin) columns (bin-strided matmul out APs are broken on HW).  Drains
  merge the two bin half-planes on-device into a 3-deep V ring (ACT copy
  of the b1 half opens plane s+1, DVE add of the b0 half finalizes plane
  s), so the output DMA ships each plane once: 6.1 MB f16 total.  A
  two-stage deferral (ABS at lag PIPE, CLAMP+matmuls+drains at lag PIPE2)
  keeps cross-engine producers off in-order queue heads, and DMA issue
  alternates the SP/ACT queues.

  TimelineSim: 110.5us/core (v7 baseline: 308.7us).  Rel L2 err 1.30e-2,
  dominated by the deterministic fp8-e3m4 rhs quantization.
"""

import os
import sys
from collections import deque

import numpy as np

sys.path.insert(0, "/opt/trn_rl_repo")

import ml_dtypes

import concourse.bass as bass
import concourse.bacc as bacc
import concourse.mybir as mybir
import concourse.tile as tile
from concourse.bass_utils import run_bass_kernel_spmd

H, W, BINS = 480, 640, 10
NSEG = BINS - 1          # 9 time segments (t*=9 folds into seg 8 with f=1)
P = 128
WY = 32                  # y-window width
WX = 32                  # x-window width
NQ32 = (H + WY - 1) // WY   # 15
NR32 = (W + WX - 1) // WX   # 20
NQ = 4                   # 128-tall psum stripes
NKEY = NSEG * NQ32 * NR32   # 2700 buckets
N_CORES = 8
G = int(os.environ.get("EVS8_G", "63"))        # tiles per batched group
CHUNK = int(os.environ.get("EVS8_CHUNK", "32"))  # rhs tiles per DMA chunk

# static engine mixes (percent)
SCAT = int(os.environ.get("EVS8_SCAT", "75"))   # share of groups built by
#                                         gpsimd local_scatter
HOSTM = int(os.environ.get("EVS8_HOSTM", "25"))  # share with host-shipped
#                                         m tiles (zero engine work; rest
#                                         of the groups use the chain)
SUB_DVE = int(os.environ.get("EVS8_SUB_DVE", "100"))     # rest -> Pool
ABS_ACT = int(os.environ.get("EVS8_ABS_ACT", "100"))
ABS_DVE = int(os.environ.get("EVS8_ABS_DVE", "0"))      # rest -> Pool
DRAIN_CA = int(os.environ.get("EVS8_DRAIN_CA", "704"))  # ACT cols of 1280
PIPE = int(os.environ.get("EVS8_PIPE", "3"))
CHB = int(os.environ.get("EVS8_CHB", "6"))
TAPER = int(os.environ.get("EVS8_TAPER", "126"))

F32 = mybir.dt.float32
F16 = mybir.dt.float16
F8 = mybir.dt.float8e3

_prog_cache: dict = {}


def _host_prep(ev):
    """Bucket one batch's events; returns (counts[NKEY], pack data)."""
    if ev.shape[0] == 0:
        ev = np.array([[0.0, 0.0, 0.25, 0.0, 0.0],
                       [0.0, 0.0, 0.75, 0.0, 0.0]], np.float32)
    x = ev[:, 0].astype(np.float64)
    y = ev[:, 1].astype(np.float64)
    t = ev[:, 2].astype(np.float64)
    p = ev[:, 3].astype(np.float32)
    t0 = t[0]
    tN = t[-1]
    denom = tN - t0
    if denom > 0:
        tp = (BINS - 1) * np.clip((t - t0) / denom, 0.0, 1.0)
    else:
        tp = np.zeros_like(t)
    s = np.clip(np.floor(tp).astype(np.int32), 0, NSEG - 1)
    f = (tp - s).astype(np.float64)

    iy = np.floor(y).astype(np.int64)
    icy = np.ceil(y).astype(np.int64)
    ix = np.floor(x).astype(np.int64)
    icx = np.ceil(x).astype(np.int64)
    qf, qc = iy // WY, icy // WY
    rf, rc = ix // WX, icx // WX
    n = len(x)
    idx0 = np.arange(n, dtype=np.int64)

    ys = qf != qc
    xs = rf != rc
    both = ys & xs
    inst_idx = np.concatenate([idx0, idx0[ys], idx0[xs], idx0[both]])
    inst_q = np.concatenate([qf, qc[ys], qf[xs], qc[both]])
    inst_r = np.concatenate([rf, rf[ys], rc[xs], rc[both]])
    key = (s[inst_idx] * NQ32 + inst_q) * NR32 + inst_r
    counts = np.bincount(key, minlength=NKEY)
    return counts, (x, y, f, p, ix, inst_idx, inst_q, inst_r, key)


def _pack_core(pack, tiles_per_key, T_tot):
    x, y, f, p, ix, inst_idx, inst_q, inst_r, key = pack
    col0 = np.zeros(NKEY + 1, np.int64)
    col0[1:] = np.cumsum(tiles_per_key)
    order = np.argsort(key, kind="stable")
    skey = key[order]
    sidx = inst_idx[order]
    sq = inst_q[order]
    sr = inst_r[order]
    group_start = np.searchsorted(skey, np.arange(NKEY))
    rank = np.arange(len(skey)) - group_start[skey]
    slot = col0[skey] * P + rank
    part = (slot % P).astype(np.int64)
    col = (slot // P).astype(np.int64)

    yrel = (y[sidx] - WY * sq).astype(np.float64)

    # schedule-derived per-tile info: group base + rhs sign (+hat for scat
    # groups, -hat for chain groups)
    (groups, route_seq, _, _, chain_base, scat_base,
     hostm_base) = _schedule(T_tot)
    g0s = np.array([g[0] for g in groups], np.int64)
    tile_g = np.searchsorted(g0s, np.arange(T_tot), side="right") - 1
    rcode = np.array([{"scat": 0, "hostm": 1, "chain": 2}[route_seq[g]]
                      for g in range(len(groups))])
    tile_sign = np.where(rcode[tile_g] == 2, -1.0, 1.0)   # [T_tot]
    tile_base = g0s[tile_g]            # group start tile of each tile
    cbase = np.array(chain_base, np.int64)
    sbase = np.array(scat_base, np.int64)
    hbase = np.array(hostm_base, np.int64)
    # compact per-tile destination columns
    last_n = groups[-1][1] - groups[-1][0]
    lr = rcode[-1]
    n_scat = max(1, int(sbase[-1] + (last_n if lr == 0 else 0)))
    n_hostm = max(1, int(hbase[-1] + (last_n if lr == 1 else 0)))
    n_chain = max(1, int(cbase[-1] + (last_n if lr == 2 else 0)))
    tile_local = np.arange(T_tot) - tile_base
    tile_ccol = cbase[tile_g] + tile_local      # chain-compact column
    tile_scol = sbase[tile_g] + tile_local      # scat-compact column
    tile_hcol = hbase[tile_g] + tile_local      # hostm-compact column

    Yc = np.zeros((P, n_chain), np.float32)
    chain_slots = rcode[tile_g[col]] == 2
    Yc[part[chain_slots], tile_ccol[col[chain_slots]]] = \
        yrel[chain_slots].astype(np.float32)

    # y taps; consumed by the scat groups (as scatter data/idx) and the
    # hostm groups (as a dense prebuilt +hat lhsT)
    cy0 = np.floor(yrel).astype(np.int64)
    fy = yrel - cy0
    YD = np.zeros((P, 2 * n_scat), np.float16)
    YI = np.full((P, 2 * n_scat), -1, np.int16)
    MH = np.zeros((P, WY * n_hostm), np.float16)
    local = (col - tile_base[col]) * WY
    scol2 = 2 * tile_scol[col]
    ss = rcode[tile_g[col]] == 0
    hh = rcode[tile_g[col]] == 1
    hcol = tile_hcol[col] * WY
    for k, (c, h) in enumerate(((cy0, 1.0 - fy), (cy0 + 1, fy))):
        ok = ss & (c >= 0) & (c < WY)
        YD[part[ss], scol2[ss] + k] = h[ss].astype(np.float16)
        YI[part[ok], scol2[ok] + k] = (local[ok] + c[ok]).astype(np.int16)
        okh = hh & (c >= 0) & (c < WY)
        MH[part[okh], hcol[okh] + c[okh]] = h[okh].astype(np.float16)

    # rhs: 64 interleaved cols (2*cx + b) = sign * 8 * p * w_b * hx[cx]
    xi = x[sidx]
    fi = f[sidx]
    pi = p[sidx].astype(np.float64)
    ixi = ix[sidx]
    s8 = 8.0 * tile_sign[col] * pi
    cf = ixi - WX * sr           # floor-tap col (may be -1 for ceil dups)
    cc = cf + 1                  # ceil-tap col (may be 32 for floor side)
    hx1 = xi - ixi               # ceil-tap weight
    hx0 = 1.0 - hx1
    w0 = s8 * (1.0 - fi)
    w1 = s8 * fi
    RHS = np.zeros((P, T_tot, 2 * WX), np.float32)
    for valid, c, h in ((cf >= 0, cf, hx0), (cc < WX, cc, hx1)):
        for b, wv in ((0, w0), (1, w1)):
            RHS[part[valid], col[valid], 2 * c[valid] + b] = \
                (h * wv)[valid].astype(np.float32)
    RHS8 = RHS.reshape(P, T_tot * 2 * WX).astype(ml_dtypes.float8_e3m4)
    return {"ev_y": Yc, "ev_rhs": RHS8, "ev_yd": YD,
            "ev_yi": YI, "ev_mh": MH}


def _mix_seq(fracs, n):
    """Maximally-even interleave (error diffusion) of engine choices."""
    tot = max(1, sum(fracs.values()))
    fr = {k: v / tot for k, v in fracs.items()}
    cnt = {k: 0 for k in fr}
    seq = []
    for i in range(n):
        pick = max(fr, key=lambda k: fr[k] * (i + 1) - cnt[k])
        cnt[pick] += 1
        seq.append(pick)
    return seq


def _schedule(T_tot):
    """Group slices + per-group route assignment, shared host/device.

    Returns (groups, route_seq, sub_seq, abs_seq, chain_base, scat_base):
    the *_base lists give each group's start offset in the compact
    chain-y / scat-data arrays (in tiles)."""
    bounds = list(range(0, max(0, T_tot - TAPER), G))
    g_small = max(8, G // 4)
    bounds += list(range(max(0, T_tot - TAPER), T_tot, g_small))
    bounds.append(T_tot)
    groups = [(a, b) for a, b in zip(bounds[:-1], bounds[1:]) if b > a]
    n = len(groups)
    route_seq = _mix_seq({"scat": SCAT, "hostm": HOSTM,
                          "chain": max(0, 100 - SCAT - HOSTM)}, n)
    nchain = sum(1 for r in route_seq if r == "chain")
    sub_seq = _mix_seq({"dve": SUB_DVE, "pool": 100 - SUB_DVE}, nchain)
    abs_seq = _mix_seq({"act": ABS_ACT, "dve": ABS_DVE,
                        "pool": max(0, 100 - ABS_ACT - ABS_DVE)}, nchain)
    chain_base, scat_base, hostm_base = [], [], []
    cb = sb = hb = 0
    for g, (a, b) in enumerate(groups):
        chain_base.append(cb)
        scat_base.append(sb)
        hostm_base.append(hb)
        if route_seq[g] == "scat":
            sb += b - a
        elif route_seq[g] == "hostm":
            hb += b - a
        else:
            cb += b - a
    return (groups, route_seq, sub_seq, abs_seq, chain_base, scat_base,
            hostm_base)


def _build_program(tiles_per_key, T_tot):
    nc = bacc.Bacc("TRN2", debug=False)
    (groups, route_seq, sub_seq, abs_seq, chain_base, scat_base,
     hostm_base) = _schedule(T_tot)
    ngroups = len(groups)
    last_n = groups[-1][1] - groups[-1][0]
    lr = route_seq[-1]
    n_scat = max(1, scat_base[-1] + (last_n if lr == "scat" else 0))
    n_hostm = max(1, hostm_base[-1] + (last_n if lr == "hostm" else 0))
    n_chain = max(1, chain_base[-1] + (last_n if lr == "chain" else 0))
    y_d = nc.dram_tensor("ev_y", [P, n_chain], F32, kind="ExternalInput")
    yd_d = nc.dram_tensor("ev_yd", [P, 2 * n_scat], F16,
                          kind="ExternalInput")
    yi_d = nc.dram_tensor("ev_yi", [P, 2 * n_scat], mybir.dt.int16,
                          kind="ExternalInput")
    rhs_d = nc.dram_tensor("ev_rhs", [P, T_tot * 2 * WX], F8,
                           kind="ExternalInput")
    mh_d = nc.dram_tensor("ev_mh", [P, WY * n_hostm], F16,
                          kind="ExternalInput")
    out_d = nc.dram_tensor("outv", [BINS, NQ, P, W], F16,
                           kind="ExternalOutput")

    col0 = np.zeros(NKEY + 1, np.int64)
    col0[1:] = np.cumsum(tiles_per_key)

    Alu = mybir.AluOpType
    Act = mybir.ActivationFunctionType

    # global tile stream: (c, sq_idx, ghat, r, first, last)
    tiles = []
    sq_last_tile = {}
    for si in range(NSEG):
        for qi in range(NQ):
            sqi = si * NQ + qi
            for q32 in range(4 * qi, min(4 * qi + 4, NQ32)):
                for r in range(NR32):
                    k = (si * NQ32 + q32) * NR32 + r
                    ntile = int(tiles_per_key[k])
                    cbase = int(col0[k])
                    for j in range(ntile):
                        tiles.append((cbase + j, sqi, q32 - 4 * qi, r,
                                      j == 0, j == ntile - 1))
                        sq_last_tile[sqi] = len(tiles) - 1
    assert len(tiles) == T_tot


    with tile.TileContext(nc) as tc:
        with (
            tc.tile_pool(name="persist", bufs=1) as persist,
            tc.tile_pool(name="psum", bufs=int(os.environ.get("EVS8_PSB", "2")), space="PSUM") as psump,
            tc.tile_pool(name="chunk", bufs=CHB) as chp,
            tc.tile_pool(name="dg", bufs=int(os.environ.get("EVS8_DB", "8"))) as dp,
            tc.tile_pool(name="zg", bufs=int(os.environ.get("EVS8_DB", "8"))) as zp,
            tc.tile_pool(name="mg", bufs=int(os.environ.get("EVS8_MB", "8"))) as mp,
            tc.tile_pool(name="mh", bufs=4) as mhp,
        ):
            dmaq = [nc.sync, nc.scalar]
            dmaqi = [0]

            def dma_rr(**kw):
                eng = dmaq[dmaqi[0] % len(dmaq)]
                dmaqi[0] += 1
                eng.dma_start(**kw)

            # --- y values (small first chunk so group 0 starts ASAP)
            yt = persist.tile([P, n_chain], F32, tag="yt")
            ydt = persist.tile([P, 2 * n_scat], F16, tag="ydt")
            yit = persist.tile([P, 2 * n_scat], mybir.dt.int16, tag="yit")
            for tot, dst, src_d, m in ((n_chain, yt, y_d, 1),
                                       (n_scat, ydt, yd_d, 2),
                                       (n_scat, yit, yi_d, 2)):
                cuts = [0, tot // 16, tot // 4, tot // 2,
                        3 * tot // 4, tot]
                for y0, y1 in zip(cuts[:-1], cuts[1:]):
                    if y1 > y0:
                        dma_rr(out=dst[:, m * y0:m * y1],
                               in_=src_d[:, m * y0:m * y1])

            # --- V ring: 3 generations x NQ q-blocks of one output plane
            VR = 3
            vring = persist.tile([P, VR * NQ * W], F16, tag="vring")

            # --- constants: io = iota cols 0..31 (f16), ACT table warm
            ioi = persist.tile([P, WY], mybir.dt.int32, tag="ioi")
            nc.gpsimd.iota(ioi[:], pattern=[[1, WY]], base=0,
                           channel_multiplier=0)
            io16 = persist.tile([P, WY], F16, tag="io16")
            nc.vector.tensor_copy(io16[:], ioi[:])
            warm = persist.tile([1, 1], F16, tag="warm")
            nc.vector.memset(warm[:], 0.0)
            nc.scalar.activation(warm[:], warm[:], Act.Abs)

            # --- rhs chunk streaming
            chunk_tiles: dict = {}

            def get_chunk(ch):
                if ch not in chunk_tiles:
                    t = chp.tile([P, CHUNK * 2 * WX], F8, tag="ch")
                    lo = ch * CHUNK * 2 * WX
                    hi = min((ch + 1) * CHUNK * 2 * WX, T_tot * 2 * WX)
                    dma_rr(out=t[:, 0:hi - lo], in_=rhs_d[:, lo:hi])
                    chunk_tiles[ch] = t
                return chunk_tiles[ch]

            for _pc in range(4):
                get_chunk(_pc)

            psum_tiles: dict = {}
            ph_q = deque()
            ph_q2 = deque()
            PIPE2 = int(os.environ.get("EVS8_PIPE2", str(2 * PIPE + 4)))

            def flush(keep):
                while len(ph_q) > keep:
                    ph_q.popleft()()

            def flush2(keep):
                while len(ph_q2) > keep:
                    ph_q2.popleft()()

            def get_psum(sqi):
                if sqi not in psum_tiles:
                    psum_tiles[sqi] = psump.tile([P, 2 * W], F32, tag="ps",
                                                 name=f"ps{sqi % int(os.environ.get('EVS8_PSB', '2'))}",
                                                 uniquify=True)
                return psum_tiles[sqi]

            def emit_mms(gtiles, mg):
                for j, (c, sqi, gh, r, first, last) in enumerate(gtiles):
                    ch, lo = divmod(c, CHUNK)
                    rhs_t = get_chunk(ch)
                    ps = get_psum(sqi)
                    nc.tensor.matmul(
                        ps[WY * gh:WY * (gh + 1),
                           2 * WX * r:2 * WX * (r + 1)],
                        lhsT=mg[:, j * WY:(j + 1) * WY],
                        rhs=rhs_t[:, lo * 2 * WX:(lo + 1) * 2 * WX],
                        start=first, stop=last,
                        tile_position=(0, WY * gh))

            def make_abs(ci, gw, dg, cell):
                def phA():
                    ae = abs_seq[ci]
                    zg = zp.tile([P, G * WY], F16, tag="zg")
                    if ae == "act":
                        nc.scalar.activation(zg[:, 0:gw], dg[:, 0:gw],
                                             Act.Abs)
                    elif ae == "dve":
                        nc.vector.scalar_tensor_tensor(
                            zg[:, 0:gw], dg[:, 0:gw], -1.0, dg[:, 0:gw],
                            op0=Alu.mult, op1=Alu.max)
                    else:
                        nc.gpsimd.scalar_tensor_tensor(
                            zg[:, 0:gw], dg[:, 0:gw], -1.0, dg[:, 0:gw],
                            op0=Alu.mult, op1=Alu.max)
                    cell["zg"] = zg

                return phA

            def make_mms(route, gtiles, gw, cell):
                def phB():
                    if route in ("scat", "hostm"):
                        mg = cell["mg"]
                    else:
                        zg = cell["zg"]
                        mg = mp.tile([P, G * WY], F16, tag="mg")
                        nc.vector.tensor_scalar(mg[:, 0:gw], zg[:, 0:gw],
                                                1.0, 0.0,
                                                op0=Alu.subtract,
                                                op1=Alu.min)
                    emit_mms(gtiles, mg)

                return phB

            def vslot(plane, qi):
                base = ((plane % 3) * NQ + qi) * W
                return vring[:, base:base + W]

            def make_drain_pieces(sqi):
                """Merge the two bin half-planes into the V ring (copy +
                add, same column count as two copies) and DMA finalized
                planes; halves output bytes vs shipping both halves."""
                si, qi = divmod(sqi, NQ)
                rows = min(P, H - P * qi)

                def pc_act():
                    ps = psum_tiles[sqi]
                    pv = ps[0:rows, :].rearrange("p (x b) -> p x b", b=2)
                    # open plane si+1 with segment si's b1 half
                    nc.scalar.activation(vslot(si + 1, qi)[0:rows, :],
                                         pv[:, :, 1], Act.Copy)

                def pc_dve():
                    ps = psum_tiles.pop(sqi)
                    pv = ps[0:rows, :].rearrange("p (x b) -> p x b", b=2)
                    vv = vslot(si, qi)[0:rows, :]
                    if si == 0:
                        nc.vector.tensor_copy(vv, pv[:, :, 0])
                    else:
                        nc.vector.scalar_tensor_tensor(
                            vv, pv[:, :, 0], 0.0, vv,
                            op0=Alu.add, op1=Alu.add)
                    # plane si's q-block is final now
                    dma_rr(out=out_d[si, qi, 0:rows, :], in_=vv)
                    if si == NSEG - 1:
                        dma_rr(out=out_d[NSEG, qi, 0:rows, :],
                               in_=vslot(NSEG, qi)[0:rows, :])

                return [pc_act, pc_dve]

            last_to_sq = {v: k for k, v in sq_last_tile.items()}
            pending_pieces = deque()
            ci = 0  # chain-group ordinal
            for gi, (g0, g1) in enumerate(groups):
                gtiles = tiles[g0:g1]
                gn = len(gtiles)
                c0 = gtiles[0][0]
                # touch psums in stream order so pool cycling stays sane
                for tl in gtiles:
                    get_psum(tl[1])
                cell = {}
                if route_seq[gi] == "hostm":
                    # lhsT shipped prebuilt from the host: one DMA, no
                    # engine work at all
                    hb = hostm_base[gi]
                    mg = mhp.tile([P, G * WY], F16, tag="mh", name="mh")
                    dma_rr(out=mg[:, 0:gn * WY],
                           in_=mh_d[:, WY * hb:WY * (hb + gn)])
                    cell["mg"] = mg
                    ph_q.append(lambda: None)
                elif route_seq[gi] == "scat":
                    # whole m tile built by one gpsimd scatter of the
                    # per-event (1-fy, fy) taps
                    sb = scat_base[gi]
                    mg = mp.tile([P, G * WY], F16, tag="mgs", name="mgs")
                    nc.gpsimd.local_scatter(
                        mg[:, 0:gn * WY], ydt[:, 2 * sb:2 * (sb + gn)],
                        yit[:, 2 * sb:2 * (sb + gn)], channels=P,
                        num_elems=gn * WY, num_idxs=2 * gn)
                    cell["mg"] = mg
                    ph_q.append(lambda: None)
                else:
                    dg = dp.tile([P, G * WY], F16, tag="dgt")
                    iob = io16[:].rearrange("p (o c) -> p o c", o=1) \
                        .to_broadcast([P, gn, WY])
                    cb = chain_base[gi]
                    yb = yt[:, cb:cb + gn] \
                        .rearrange("p (g o) -> p g o", o=1) \
                        .to_broadcast([P, gn, WY])
                    dgv = dg[:, 0:gn * WY].rearrange("p (g c) -> p g c",
                                                     g=gn)
                    if sub_seq[ci] == "dve":
                        nc.vector.tensor_tensor(dgv, iob, yb,
                                                op=Alu.subtract)
                    else:
                        nc.gpsimd.tensor_tensor(dgv, iob, yb,
                                                op=Alu.subtract)
                    ph_q.append(make_abs(ci, gn * WY, dg, cell))
                    ci += 1
                ph_q2.append(make_mms(route_seq[gi], gtiles, gn * WY, cell))
                # drains queue behind the completing group's matmuls
                for ti in range(g0, g0 + gn):
                    if ti in last_to_sq:
                        for pc in make_drain_pieces(last_to_sq[ti]):
                            ph_q2.append(pc)
                flush(PIPE)
                flush2(PIPE2)
            flush(0)
            flush2(0)
    nc.finalize()
    return nc


def kernel(events, lengths):
    events = np.ascontiguousarray(events, dtype=np.float32)
    lengths = np.asarray(lengths)
    B = int(lengths.shape[0])
    offs = np.zeros(B + 1, np.int64)
    offs[1:] = np.cumsum(lengths)

    packs = []
    counts = np.zeros((B, NKEY), np.int64)
    for bi in range(B):
        c, pk = _host_prep(events[offs[bi]:offs[bi + 1]])
        counts[bi] = c
        packs.append(pk)

    tiles_per_key = np.maximum(1, -(-counts.max(axis=0) // P)).astype(np.int64)
    T_tot = int(tiles_per_key.sum())

    key = (tuple(tiles_per_key.tolist()), T_tot, G, CHUNK, SCAT, TAPER,
           HOSTM, SUB_DVE, ABS_ACT, ABS_DVE, DRAIN_CA, PIPE,
           os.environ.get("EVS8_PIPE2", ""),
           os.environ.get("EVS8_PSB", "2"),
           os.environ.get("EVS8_DB", "8"), os.environ.get("EVS8_MB", "8"))
    if key not in _prog_cache:
        _prog_cache[key] = _build_program(tiles_per_key, T_tot)
    nc = _prog_cache[key]

    in_maps = [_pack_core(pk, tiles_per_key, T_tot) for pk in packs]
    trace = bool(int(os.environ.get("EVS_TRACE", "0")))
    res = run_bass_kernel_spmd(nc, in_maps, core_ids=list(range(B)),
                               trace=trace)
    global last_results
    last_results = res

    out = np.zeros((B, BINS, H, W), np.float32)
    for bi in range(B):
        ov = np.asarray(res.results[bi]["outv"]).astype(np.float32)
        out[bi] = ov.reshape(BINS, NQ * P, W)[:, :H] * (1.0 / 8.0)
    return out


last_results = None


if __name__ == "__main__":
    # tiny smoke test with synthetic events
    rng = np.random.default_rng(0)
    B0, NP0 = 8, 2000
    N0 = B0 * NP0
    x = rng.uniform(0, W - 1, N0).astype(np.float32)
    y = rng.uniform(0, H - 1, N0).astype(np.float32)
    t = np.sort(rng.uniform(0, 1, (B0, NP0)).astype(np.float32), axis=1).ravel()
    p = (2.0 * rng.integers(0, 2, N0) - 1).astype(np.float32)
    b = np.repeat(np.arange(B0), NP0).astype(np.float32)
    ev = np.stack([x, y, t, p, b], axis=1)
    ln = np.full(B0, NP0, np.int32)
    out = kernel(ev, ln)
    # numpy reference
    ref = np.zeros((B0, BINS, H, W), np.float64)
    for bi in range(B0):
        sl = slice(bi * NP0, (bi + 1) * NP0)
        xx, yy, tt2, pp = x[sl], y[sl], t[sl], p[sl]
        t0, tN = tt2[0], tt2[-1]
        ts = (BINS - 1) * np.clip((tt2 - t0) / (tN - t0), 0, 1)
        import itertools
        for xr_f, yr_f, br_f in itertools.product([np.floor, np.ceil], repeat=3):
            xr, yr, br = xr_f(xx), yr_f(yy), br_f(ts)
            valid = (((xr != xx) | (xr_f is np.floor))
                     & ((yr != yy) | (yr_f is np.floor))
                     & ((br != ts) | (br_f is np.floor))
                     & (xr < W) & (yr < H) & (br < BINS))
            kb = lambda a_: np.maximum(0, 1 - np.abs(a_))
            val = np.where(valid, pp * kb(xr - xx) * kb(yr - yy) * kb(br - ts), 0)
            np.add.at(ref[bi].ravel(),
                      np.where(valid, (xr + yr * W + br * H * W).astype(np.int64), 0),
                      val)
    num = np.linalg.norm((out - ref).ravel())
    den = np.linalg.norm(ref.ravel())
    print("smoke rel l2 err:", num / max(den, 1e-30))
    print("smoke max abs err:", np.abs(out - ref).max())


# revision 48
# speedup vs baseline: 1.0118x; 1.0118x over previous
"""EventVolumeSurface trilinear voxel-grid kernel for Trainium2 (Bass/Tile).

v10 strategy (data-parallel over batch, 1 batch -> 1 NeuronCore):
  Events are bucketed by (time-segment s in [0,9), y-window q32 = iy>>5 in
  [0,15), x-window r32 = ix>>5 in [0,20)) with straddle duplication at the
  32-boundaries.  Tiles of 128 event slots stream in bucket order, batched
  into groups of G=63.

  Host ships, per 128-event tile slot:
    - rhs [128, T*64] fp8 e3m4: 64 interleaved columns (2*cx + b) holding
      sign * 8 * p * w_b * hx[cx]; hx = 2-tap x hat, w_0 = 1-f, w_1 = f
      (time-bin weights); x8 keeps values in e3m4's normal range (host
      divides by 8 at unshard); sign matches the group route below.
    - scat groups (75%): per-slot y taps (1-fy, fy) f16 + int16
      group-local scatter indices (-1 = skip, handles window straddle).
    - hostm groups (25%): the +hat lhsT tile prebuilt on the host (f16).
    - chain groups (0% by default, kept as a tuning fallback): yhat f32.

  Device m-tile build (columns cost engine time; partitions are free):
    scat:  ONE gpsimd local_scatter builds the whole [128, 63*32] +hat
           lhsT group on Pool (~47ns/tile, replaces sub/abs/min chains)
    hostm: one DMA, zero engine work (spends spare DMA bandwidth to
           relieve Pool; the 75/25 split equalizes Pool vs DMA)
    chain: DVE broadcast-subtract -> ACT Abs -> DVE fused min (= -hat)
    MM: psum[32*g:+32, 64*r:+64] += m_j^T @ rhs_j  (f16 x fp8e3, 64 out
        cols ~36ns; tile_position puts the 32 out rows at partition 32*g)
  PSUM is one [128, 1280] f32 tile per (s, q128) stripe with interleaved
  (x, bin) columns (bin-strided matmul out APs are broken on HW).  Drains
  merge the two bin half-planes on-device into a 3-deep V ring (ACT copy
  of the b1 half opens plane s+1, DVE add of the b0 half finalizes plane
  s), so the output DMA ships each plane once: 6.1 MB f16 total.  A
  two-stage deferral (ABS at lag PIPE, CLAMP+matmuls+drains at lag PIPE2)
  keeps cross-engine producers off in-order queue heads, and DMA issue
  alternates the SP/ACT queues.

  TimelineSim: 110.5us/core (v7 baseline: 308.7us).  Rel L2 err 1.30e-2,
  dominated by the deterministic fp8-e3m4 rhs quantization.
"""

import os
import sys
from collections import deque

import numpy as np

sys.path.insert(0, "/opt/trn_rl_repo")

import ml_dtypes

import concourse.bass as bass
import concourse.bacc as bacc
import concourse.mybir as mybir
import concourse.tile as tile
from concourse.bass_utils import run_bass_kernel_spmd

H, W, BINS = 480, 640, 10
NSEG = BINS - 1          # 9 time segments (t*=9 folds into seg 8 with f=1)
P = 128
WY = 32                  # y-window width
WX = 32                  # x-window width
NQ32 = (H + WY - 1) // WY   # 15
NR32 = (W + WX - 1) // WX   # 20
NQ = 4                   # 128-tall psum stripes
NKEY = NSEG * NQ32 * NR32   # 2700 buckets
N_CORES = 8
G = int(os.environ.get("EVS8_G", "63"))        # tiles per batched group
CHUNK = int(os.environ.get("EVS8_CHUNK", "32"))  # rhs tiles per DMA chunk

# static engine mixes (percent)
SCAT = int(os.environ.get("EVS8_SCAT", "75"))   # share of groups built by
#                                         gpsimd local_scatter
HOSTM = int(os.environ.get("EVS8_HOSTM", "25"))  # share with host-shipped
#                                         m tiles (zero engine work; rest
#                                         of the groups use the chain)
SUB_DVE = int(os.environ.get("EVS8_SUB_DVE", "100"))     # rest -> Pool
ABS_ACT = int(os.environ.get("EVS8_ABS_ACT", "100"))
ABS_DVE = int(os.environ.get("EVS8_ABS_DVE", "0"))      # rest -> Pool
DRAIN_CA = int(os.environ.get("EVS8_DRAIN_CA", "704"))  # ACT cols of 1280
PIPE = int(os.environ.get("EVS8_PIPE", "3"))
CHB = int(os.environ.get("EVS8_CHB", "6"))
TAPER = int(os.environ.get("EVS8_TAPER", "126"))

F32 = mybir.dt.float32
F16 = mybir.dt.float16
F8 = mybir.dt.float8e3

_prog_cache: dict = {}


def _host_prep(ev):
    """Bucket one batch's events; returns (counts[NKEY], pack data)."""
    if ev.shape[0] == 0:
        ev = np.array([[0.0, 0.0, 0.25, 0.0, 0.0],
                       [0.0, 0.0, 0.75, 0.0, 0.0]], np.float32)
    x = ev[:, 0].astype(np.float64)
    y = ev[:, 1].astype(np.float64)
    t = ev[:, 2].astype(np.float64)
    p = ev[:, 3].astype(np.float32)
    t0 = t[0]
    tN = t[-1]
    denom = tN - t0
    if denom > 0:
        tp = (BINS - 1) * np.clip((t - t0) / denom, 0.0, 1.0)
    else:
        tp = np.zeros_like(t)
    s = np.clip(np.floor(tp).astype(np.int32), 0, NSEG - 1)
    f = (tp - s).astype(np.float64)

    iy = np.floor(y).astype(np.int64)
    icy = np.ceil(y).astype(np.int64)
    ix = np.floor(x).astype(np.int64)
    icx = np.ceil(x).astype(np.int64)
    qf, qc = iy // WY, icy // WY
    rf, rc = ix // WX, icx // WX
    n = len(x)
    idx0 = np.arange(n, dtype=np.int64)

    ys = qf != qc
    xs = rf != rc
    both = ys & xs
    inst_idx = np.concatenate([idx0, idx0[ys], idx0[xs], idx0[both]])
    inst_q = np.concatenate([qf, qc[ys], qf[xs], qc[both]])
    inst_r = np.concatenate([rf, rf[ys], rc[xs], rc[both]])
    key = (s[inst_idx] * NQ32 + inst_q) * NR32 + inst_r
    counts = np.bincount(key, minlength=NKEY)
    return counts, (x, y, f, p, ix, inst_idx, inst_q, inst_r, key)


def _pack_core(pack, tiles_per_key, T_tot):
    x, y, f, p, ix, inst_idx, inst_q, inst_r, key = pack
    col0 = np.zeros(NKEY + 1, np.int64)
    col0[1:] = np.cumsum(tiles_per_key)
    order = np.argsort(key, kind="stable")
    skey = key[order]
    sidx = inst_idx[order]
    sq = inst_q[order]
    sr = inst_r[order]
    group_start = np.searchsorted(skey, np.arange(NKEY))
    rank = np.arange(len(skey)) - group_start[skey]
    slot = col0[skey] * P + rank
    part = (slot % P).astype(np.int64)
    col = (slot // P).astype(np.int64)

    yrel = (y[sidx] - WY * sq).astype(np.float64)

    # schedule-derived per-tile info: group base + rhs sign (+hat for scat
    # groups, -hat for chain groups)
    (groups, route_seq, _, _, chain_base, scat_base,
     hostm_base) = _schedule(T_tot)
    g0s = np.array([g[0] for g in groups], np.int64)
    tile_g = np.searchsorted(g0s, np.arange(T_tot), side="right") - 1
    rcode = np.array([{"scat": 0, "hostm": 1, "chain": 2}[route_seq[g]]
                      for g in range(len(groups))])
    tile_sign = np.where(rcode[tile_g] == 2, -1.0, 1.0)   # [T_tot]
    tile_base = g0s[tile_g]            # group start tile of each tile
    cbase = np.array(chain_base, np.int64)
    sbase = np.array(scat_base, np.int64)
    hbase = np.array(hostm_base, np.int64)
    # compact per-tile destination columns
    last_n = groups[-1][1] - groups[-1][0]
    lr = rcode[-1]
    n_scat = max(1, int(sbase[-1] + (last_n if lr == 0 else 0)))
    n_hostm = max(1, int(hbase[-1] + (last_n if lr == 1 else 0)))
    n_chain = max(1, int(cbase[-1] + (last_n if lr == 2 else 0)))
    tile_local = np.arange(T_tot) - tile_base
    tile_ccol = cbase[tile_g] + tile_local      # chain-compact column
    tile_scol = sbase[tile_g] + tile_local      # scat-compact column
    tile_hcol = hbase[tile_g] + tile_local      # hostm-compact column

    Yc = np.zeros((P, n_chain), np.float32)
    chain_slots = rcode[tile_g[col]] == 2
    Yc[part[chain_slots], tile_ccol[col[chain_slots]]] = \
        yrel[chain_slots].astype(np.float32)

    # y taps; consumed by the scat groups (as scatter data/idx) and the
    # hostm groups (as a dense prebuilt +hat lhsT)
    cy0 = np.floor(yrel).astype(np.int64)
    fy = yrel - cy0
    YD = np.zeros((P, 2 * n_scat), np.float16)
    YI = np.full((P, 2 * n_scat), -1, np.int16)
    MH = np.zeros((P, WY * n_hostm), ml_dtypes.float8_e3m4)
    local = (col - tile_base[col]) * WY
    scol2 = 2 * tile_scol[col]
    ss = rcode[tile_g[col]] == 0
    hh = rcode[tile_g[col]] == 1
    hcol = tile_hcol[col] * WY
    for k, (c, h) in enumerate(((cy0, 1.0 - fy), (cy0 + 1, fy))):
        ok = ss & (c >= 0) & (c < WY)
        YD[part[ss], scol2[ss] + k] = h[ss].astype(np.float16)
        YI[part[ok], scol2[ok] + k] = (local[ok] + c[ok]).astype(np.int16)
        okh = hh & (c >= 0) & (c < WY)
        MH[part[okh], hcol[okh] + c[okh]] = \
            h[okh].astype(ml_dtypes.float8_e3m4)

    # rhs: 64 interleaved cols (2*cx + b) = sign * 8 * p * w_b * hx[cx]
    xi = x[sidx]
    fi = f[sidx]
    pi = p[sidx].astype(np.float64)
    ixi = ix[sidx]
    s8 = 8.0 * tile_sign[col] * pi
    cf = ixi - WX * sr           # floor-tap col (may be -1 for ceil dups)
    cc = cf + 1                  # ceil-tap col (may be 32 for floor side)
    hx1 = xi - ixi               # ceil-tap weight
    hx0 = 1.0 - hx1
    w0 = s8 * (1.0 - fi)
    w1 = s8 * fi
    RHS = np.zeros((P, T_tot, 2 * WX), np.float32)
    for valid, c, h in ((cf >= 0, cf, hx0), (cc < WX, cc, hx1)):
        for b, wv in ((0, w0), (1, w1)):
            RHS[part[valid], col[valid], 2 * c[valid] + b] = \
                (h * wv)[valid].astype(np.float32)
    RHS8 = RHS.reshape(P, T_tot * 2 * WX).astype(ml_dtypes.float8_e3m4)
    return {"ev_y": Yc, "ev_rhs": RHS8, "ev_yd": YD,
            "ev_yi": YI, "ev_mh": MH}


def _mix_seq(fracs, n):
    """Maximally-even interleave (error diffusion) of engine choices."""
    tot = max(1, sum(fracs.values()))
    fr = {k: v / tot for k, v in fracs.items()}
    cnt = {k: 0 for k in fr}
    seq = []
    for i in range(n):
        pick = max(fr, key=lambda k: fr[k] * (i + 1) - cnt[k])
        cnt[pick] += 1
        seq.append(pick)
    return seq


def _schedule(T_tot):
    """Group slices + per-group route assignment, shared host/device.

    Returns (groups, route_seq, sub_seq, abs_seq, chain_base, scat_base):
    the *_base lists give each group's start offset in the compact
    chain-y / scat-data arrays (in tiles)."""
    bounds = list(range(0, max(0, T_tot - TAPER), G))
    g_small = max(8, G // 4)
    bounds += list(range(max(0, T_tot - TAPER), T_tot, g_small))
    bounds.append(T_tot)
    groups = [(a, b) for a, b in zip(bounds[:-1], bounds[1:]) if b > a]
    n = len(groups)
    route_seq = _mix_seq({"scat": SCAT, "hostm": HOSTM,
                          "chain": max(0, 100 - SCAT - HOSTM)}, n)
    nchain = sum(1 for r in route_seq if r == "chain")
    sub_seq = _mix_seq({"dve": SUB_DVE, "pool": 100 - SUB_DVE}, nchain)
    abs_seq = _mix_seq({"act": ABS_ACT, "dve": ABS_DVE,
                        "pool": max(0, 100 - ABS_ACT - ABS_DVE)}, nchain)
    chain_base, scat_base, hostm_base = [], [], []
    cb = sb = hb = 0
    for g, (a, b) in enumerate(groups):
        chain_base.append(cb)
        scat_base.append(sb)
        hostm_base.append(hb)
        if route_seq[g] == "scat":
            sb += b - a
        elif route_seq[g] == "hostm":
            hb += b - a
        else:
            cb += b - a
    return (groups, route_seq, sub_seq, abs_seq, chain_base, scat_base,
            hostm_base)


def _build_program(tiles_per_key, T_tot):
    nc = bacc.Bacc("TRN2", debug=False)
    (groups, route_seq, sub_seq, abs_seq, chain_base, scat_base,
     hostm_base) = _schedule(T_tot)
    ngroups = len(groups)
    last_n = groups[-1][1] - groups[-1][0]
    lr = route_seq[-1]
    n_scat = max(1, scat_base[-1] + (last_n if lr == "scat" else 0))
    n_hostm = max(1, hostm_base[-1] + (last_n if lr == "hostm" else 0))
    n_chain = max(1, chain_base[-1] + (last_n if lr == "chain" else 0))
    y_d = nc.dram_tensor("ev_y", [P, n_chain], F32, kind="ExternalInput")
    yd_d = nc.dram_tensor("ev_yd", [P, 2 * n_scat], F16,
                          kind="ExternalInput")
    yi_d = nc.dram_tensor("ev_yi", [P, 2 * n_scat], mybir.dt.int16,
                          kind="ExternalInput")
    rhs_d = nc.dram_tensor("ev_rhs", [P, T_tot * 2 * WX], F8,
                           kind="ExternalInput")
    mh_d = nc.dram_tensor("ev_mh", [P, WY * n_hostm], F8,
                          kind="ExternalInput")
    out_d = nc.dram_tensor("outv", [BINS, NQ, P, W], F16,
                           kind="ExternalOutput")

    col0 = np.zeros(NKEY + 1, np.int64)
    col0[1:] = np.cumsum(tiles_per_key)

    Alu = mybir.AluOpType
    Act = mybir.ActivationFunctionType

    # global tile stream: (c, sq_idx, ghat, r, first, last)
    tiles = []
    sq_last_tile = {}
    for si in range(NSEG):
        for qi in range(NQ):
            sqi = si * NQ + qi
            for q32 in range(4 * qi, min(4 * qi + 4, NQ32)):
                for r in range(NR32):
                    k = (si * NQ32 + q32) * NR32 + r
                    ntile = int(tiles_per_key[k])
                    cbase = int(col0[k])
                    for j in range(ntile):
                        tiles.append((cbase + j, sqi, q32 - 4 * qi, r,
                                      j == 0, j == ntile - 1))
                        sq_last_tile[sqi] = len(tiles) - 1
    assert len(tiles) == T_tot


    with tile.TileContext(nc) as tc:
        with (
            tc.tile_pool(name="persist", bufs=1) as persist,
            tc.tile_pool(name="psum", bufs=int(os.environ.get("EVS8_PSB", "2")), space="PSUM") as psump,
            tc.tile_pool(name="chunk", bufs=CHB) as chp,
            tc.tile_pool(name="dg", bufs=int(os.environ.get("EVS8_DB", "8"))) as dp,
            tc.tile_pool(name="zg", bufs=int(os.environ.get("EVS8_DB", "8"))) as zp,
            tc.tile_pool(name="mg", bufs=int(os.environ.get("EVS8_MB", "8"))) as mp,
            tc.tile_pool(name="mh", bufs=4) as mhp,
        ):
            dmaq = [nc.sync, nc.scalar]
            dmaqi = [0]

            def dma_rr(**kw):
                eng = dmaq[dmaqi[0] % len(dmaq)]
                dmaqi[0] += 1
                eng.dma_start(**kw)

            # --- y values (small first chunk so group 0 starts ASAP)
            yt = persist.tile([P, n_chain], F32, tag="yt")
            ydt = persist.tile([P, 2 * n_scat], F16, tag="ydt")
            yit = persist.tile([P, 2 * n_scat], mybir.dt.int16, tag="yit")
            for tot, dst, src_d, m in ((n_chain, yt, y_d, 1),
                                       (n_scat, ydt, yd_d, 2),
                                       (n_scat, yit, yi_d, 2)):
                cuts = [0, tot // 16, tot // 4, tot // 2,
                        3 * tot // 4, tot]
                for y0, y1 in zip(cuts[:-1], cuts[1:]):
                    if y1 > y0:
                        dma_rr(out=dst[:, m * y0:m * y1],
                               in_=src_d[:, m * y0:m * y1])

            # --- V ring: 3 generations x NQ q-blocks of one output plane
            VR = 3
            vring = persist.tile([P, VR * NQ * W], F16, tag="vring")

            # --- constants: io = iota cols 0..31 (f16), ACT table warm
            ioi = persist.tile([P, WY], mybir.dt.int32, tag="ioi")
            nc.gpsimd.iota(ioi[:], pattern=[[1, WY]], base=0,
                           channel_multiplier=0)
            io16 = persist.tile([P, WY], F16, tag="io16")
            nc.vector.tensor_copy(io16[:], ioi[:])
            warm = persist.tile([1, 1], F16, tag="warm")
            nc.vector.memset(warm[:], 0.0)
            nc.scalar.activation(warm[:], warm[:], Act.Abs)

            # --- rhs chunk streaming
            chunk_tiles: dict = {}

            def get_chunk(ch):
                if ch not in chunk_tiles:
                    t = chp.tile([P, CHUNK * 2 * WX], F8, tag="ch")
                    lo = ch * CHUNK * 2 * WX
                    hi = min((ch + 1) * CHUNK * 2 * WX, T_tot * 2 * WX)
                    dma_rr(out=t[:, 0:hi - lo], in_=rhs_d[:, lo:hi])
                    chunk_tiles[ch] = t
                return chunk_tiles[ch]

            for _pc in range(4):
                get_chunk(_pc)

            psum_tiles: dict = {}
            ph_q = deque()
            ph_q2 = deque()
            PIPE2 = int(os.environ.get("EVS8_PIPE2", str(2 * PIPE + 4)))

            def flush(keep):
                while len(ph_q) > keep:
                    ph_q.popleft()()

            def flush2(keep):
                while len(ph_q2) > keep:
                    ph_q2.popleft()()

            def get_psum(sqi):
                if sqi not in psum_tiles:
                    psum_tiles[sqi] = psump.tile([P, 2 * W], F32, tag="ps",
                                                 name=f"ps{sqi % int(os.environ.get('EVS8_PSB', '2'))}",
                                                 uniquify=True)
                return psum_tiles[sqi]

            def emit_mms(gtiles, mg):
                for j, (c, sqi, gh, r, first, last) in enumerate(gtiles):
                    ch, lo = divmod(c, CHUNK)
                    rhs_t = get_chunk(ch)
                    ps = get_psum(sqi)
                    nc.tensor.matmul(
                        ps[WY * gh:WY * (gh + 1),
                           2 * WX * r:2 * WX * (r + 1)],
                        lhsT=mg[:, j * WY:(j + 1) * WY],
                        rhs=rhs_t[:, lo * 2 * WX:(lo + 1) * 2 * WX],
                        start=first, stop=last,
                        tile_position=(0, WY * gh))

            def make_abs(ci, gw, dg, cell):
                def phA():
                    ae = abs_seq[ci]
                    zg = zp.tile([P, G * WY], F16, tag="zg")
                    if ae == "act":
                        nc.scalar.activation(zg[:, 0:gw], dg[:, 0:gw],
                                             Act.Abs)
                    elif ae == "dve":
                        nc.vector.scalar_tensor_tensor(
                            zg[:, 0:gw], dg[:, 0:gw], -1.0, dg[:, 0:gw],
                            op0=Alu.mult, op1=Alu.max)
                    else:
                        nc.gpsimd.scalar_tensor_tensor(
                            zg[:, 0:gw], dg[:, 0:gw], -1.0, dg[:, 0:gw],
                            op0=Alu.mult, op1=Alu.max)
                    cell["zg"] = zg

                return phA

            def make_mms(route, gtiles, gw, cell):
                def phB():
                    if route in ("scat", "hostm"):
                        mg = cell["mg"]
                    else:
                        zg = cell["zg"]
                        mg = mp.tile([P, G * WY], F16, tag="mg")
                        nc.vector.tensor_scalar(mg[:, 0:gw], zg[:, 0:gw],
                                                1.0, 0.0,
                                                op0=Alu.subtract,
                                                op1=Alu.min)
                    emit_mms(gtiles, mg)

                return phB

            def vslot(plane, qi):
                base = ((plane % 3) * NQ + qi) * W
                return vring[:, base:base + W]

            def make_drain_pieces(sqi):
                """Merge the two bin half-planes into the V ring (copy +
                add, same column count as two copies) and DMA finalized
                planes; halves output bytes vs shipping both halves."""
                si, qi = divmod(sqi, NQ)
                rows = min(P, H - P * qi)

                def pc_act():
                    ps = psum_tiles[sqi]
                    pv = ps[0:rows, :].rearrange("p (x b) -> p x b", b=2)
                    # open plane si+1 with segment si's b1 half
                    nc.scalar.activation(vslot(si + 1, qi)[0:rows, :],
                                         pv[:, :, 1], Act.Copy)

                def pc_dve():
                    ps = psum_tiles.pop(sqi)
                    pv = ps[0:rows, :].rearrange("p (x b) -> p x b", b=2)
                    vv = vslot(si, qi)[0:rows, :]
                    if si == 0:
                        nc.vector.tensor_copy(vv, pv[:, :, 0])
                    else:
                        nc.vector.scalar_tensor_tensor(
                            vv, pv[:, :, 0], 0.0, vv,
                            op0=Alu.add, op1=Alu.add)
                    # plane si's q-block is final now
                    dma_rr(out=out_d[si, qi, 0:rows, :], in_=vv)
                    if si == NSEG - 1:
                        dma_rr(out=out_d[NSEG, qi, 0:rows, :],
                               in_=vslot(NSEG, qi)[0:rows, :])

                return [pc_act, pc_dve]

            last_to_sq = {v: k for k, v in sq_last_tile.items()}
            pending_pieces = deque()
            ci = 0  # chain-group ordinal
            for gi, (g0, g1) in enumerate(groups):
                gtiles = tiles[g0:g1]
                gn = len(gtiles)
                c0 = gtiles[0][0]
                # touch psums in stream order so pool cycling stays sane
                for tl in gtiles:
                    get_psum(tl[1])
                cell = {}
                if route_seq[gi] == "hostm":
                    # lhsT shipped prebuilt from the host: one DMA, no
                    # engine work at all
                    hb = hostm_base[gi]
                    mg = mhp.tile([P, G * WY], F8, tag="mh", name="mh")
                    dma_rr(out=mg[:, 0:gn * WY],
                           in_=mh_d[:, WY * hb:WY * (hb + gn)])
                    cell["mg"] = mg
                    ph_q.append(lambda: None)
                elif route_seq[gi] == "scat":
                    # whole m tile built by one gpsimd scatter of the
                    # per-event (1-fy, fy) taps
                    sb = scat_base[gi]
                    mg = mp.tile([P, G * WY], F16, tag="mgs", name="mgs")
                    nc.gpsimd.local_scatter(
                        mg[:, 0:gn * WY], ydt[:, 2 * sb:2 * (sb + gn)],
                        yit[:, 2 * sb:2 * (sb + gn)], channels=P,
                        num_elems=gn * WY, num_idxs=2 * gn)
                    cell["mg"] = mg
                    ph_q.append(lambda: None)
                else:
                    dg = dp.tile([P, G * WY], F16, tag="dgt")
                    iob = io16[:].rearrange("p (o c) -> p o c", o=1) \
                        .to_broadcast([P, gn, WY])
                    cb = chain_base[gi]
                    yb = yt[:, cb:cb + gn] \
                        .rearrange("p (g o) -> p g o", o=1) \
                        .to_broadcast([P, gn, WY])
                    dgv = dg[:, 0:gn * WY].rearrange("p (g c) -> p g c",
                                                     g=gn)
                    if sub_seq[ci] == "dve":
                        nc.vector.tensor_tensor(dgv, iob, yb,
                                                op=Alu.subtract)
                    else:
                        nc.gpsimd.tensor_tensor(dgv, iob, yb,
                                                op=Alu.subtract)
                    ph_q.append(make_abs(ci, gn * WY, dg, cell))
                    ci += 1
                ph_q2.append(make_mms(route_seq[gi], gtiles, gn * WY, cell))
                # drains queue behind the completing group's matmuls
                for ti in range(g0, g0 + gn):
                    if ti in last_to_sq:
                        for pc in make_drain_pieces(last_to_sq[ti]):
                            ph_q2.append(pc)
                flush(PIPE)
                flush2(PIPE2)
            flush(0)
            flush2(0)
    nc.finalize()
    return nc


def kernel(events, lengths):
    events = np.ascontiguousarray(events, dtype=np.float32)
    lengths = np.asarray(lengths)
    B = int(lengths.shape[0])
    offs = np.zeros(B + 1, np.int64)
    offs[1:] = np.cumsum(lengths)

    packs = []
    counts = np.zeros((B, NKEY), np.int64)
    for bi in range(B):
        c, pk = _host_prep(events[offs[bi]:offs[bi + 1]])
        counts[bi] = c
        packs.append(pk)

    tiles_per_key = np.maximum(1, -(-counts.max(axis=0) // P)).astype(np.int64)
    T_tot = int(tiles_per_key.sum())

    key = (tuple(tiles_per_key.tolist()), T_tot, G, CHUNK, SCAT, TAPER,
           HOSTM, SUB_DVE, ABS_ACT, ABS_DVE, DRAIN_CA, PIPE,
           os.environ.get("EVS8_PIPE2", ""),
           os.environ.get("EVS8_PSB", "2"),
           os.environ.get("EVS8_DB", "8"), os.environ.get("EVS8_MB", "8"))
    if key not in _prog_cache:
        _prog_cache[key] = _build_program(tiles_per_key, T_tot)
    nc = _prog_cache[key]

    in_maps = [_pack_core(pk, tiles_per_key, T_tot) for pk in packs]
    trace = bool(int(os.environ.get("EVS_TRACE", "0")))
    res = run_bass_kernel_spmd(nc, in_maps, core_ids=list(range(B)),
                               trace=trace)
    global last_results
    last_results = res

    out = np.zeros((B, BINS, H, W), np.float32)
    for bi in range(B):
        ov = np.asarray(res.results[bi]["outv"]).astype(np.float32)
        out[bi] = ov.reshape(BINS, NQ * P, W)[:, :H] * (1.0 / 8.0)
    return out


last_results = None


if __name__ == "__main__":
    # tiny smoke test with synthetic events
    rng = np.random.default_rng(0)
    B0, NP0 = 8, 2000
    N0 = B0 * NP0
    x = rng.uniform(0, W - 1, N0).astype(np.float32)
    y = rng.uniform(0, H - 1, N0).astype(np.float32)
    t = np.sort(rng.uniform(0, 1, (B0, NP0)).astype(np.float32), axis=1).ravel()
    p = (2.0 * rng.integers(0, 2, N0) - 1).astype(np.float32)
    b = np.repeat(np.arange(B0), NP0).astype(np.float32)
    ev = np.stack([x, y, t, p, b], axis=1)
    ln = np.full(B0, NP0, np.int32)
    out = kernel(ev, ln)
    # numpy reference
    ref = np.zeros((B0, BINS, H, W), np.float64)
    for bi in range(B0):
        sl = slice(bi * NP0, (bi + 1) * NP0)
        xx, yy, tt2, pp = x[sl], y[sl], t[sl], p[sl]
        t0, tN = tt2[0], tt2[-1]
        ts = (BINS - 1) * np.clip((tt2 - t0) / (tN - t0), 0, 1)
        import itertools
        for xr_f, yr_f, br_f in itertools.product([np.floor, np.ceil], repeat=3):
            xr, yr, br = xr_f(xx), yr_f(yy), br_f(ts)
            valid = (((xr != xx) | (xr_f is np.floor))
                     & ((yr != yy) | (yr_f is np.floor))
                     & ((br != ts) | (br_f is np.floor))
                     & (xr < W) & (yr < H) & (br < BINS))
            kb = lambda a_: np.maximum(0, 1 - np.abs(a_))
            val = np.where(valid, pp * kb(xr - xx) * kb(yr - yy) * kb(br - ts), 0)
            np.add.at(ref[bi].ravel(),
                      np.where(valid, (xr + yr * W + br * H * W).astype(np.int64), 0),
                      val)
    num = np.linalg.norm((out - ref).ravel())
    den = np.linalg.norm(ref.ravel())
    print("smoke rel l2 err:", num / max(den, 1e-30))
    print("smoke max abs err:", np.abs(out - ref).max())


# revision 49
# speedup vs baseline: 1.0330x; 1.0209x over previous
"""EventVolumeSurface trilinear voxel-grid kernel for Trainium2 (Bass/Tile).

v10 strategy (data-parallel over batch, 1 batch -> 1 NeuronCore):
  Events are bucketed by (time-segment s in [0,9), y-window q32 = iy>>5 in
  [0,15), x-window r32 = ix>>5 in [0,20)) with straddle duplication at the
  32-boundaries.  Tiles of 128 event slots stream in bucket order, batched
  into groups of G=63.

  Host ships, per 128-event tile slot:
    - rhs [128, T*64] fp8 e3m4: 64 interleaved columns (2*cx + b) holding
      sign * 8 * p * w_b * hx[cx]; hx = 2-tap x hat, w_0 = 1-f, w_1 = f
      (time-bin weights); x8 keeps values in e3m4's normal range (host
      divides by 8 at unshard); sign matches the group route below.
    - scat groups (75%): per-slot y taps (1-fy, fy) f16 + int16
      group-local scatter indices (-1 = skip, handles window straddle).
    - hostm groups (28%): the +hat lhsT tile prebuilt on the host, in
      fp8 e3m4 (unscaled hat values, so no psum-scale mixing).
    - chain groups (0% by default, kept as a tuning fallback): yhat f32.

  Device m-tile build (columns cost engine time; partitions are free):
    scat:  ONE gpsimd local_scatter builds the whole [128, 63*32] +hat
           lhsT group on Pool (~47ns/tile, replaces sub/abs/min chains)
    hostm: one DMA, zero engine work (spends spare DMA bandwidth to
           relieve Pool; the 75/25 split equalizes Pool vs DMA)
    chain: DVE broadcast-subtract -> ACT Abs -> DVE fused min (= -hat)
    MM: psum[32*g:+32, 64*r:+64] += m_j^T @ rhs_j  (f16 x fp8e3, 64 out
        cols ~36ns; tile_position puts the 32 out rows at partition 32*g)
  PSUM is one [128, 1280] f32 tile per (s, q128) stripe with interleaved
  (x, bin) columns (bin-strided matmul out APs are broken on HW).  Drains
  merge the two bin half-planes on-device into a 3-deep V ring (ACT copy
  of the b1 half opens plane s+1, DVE add of the b0 half finalizes plane
  s), so the output DMA ships each plane once: 6.1 MB f16 total.  A
  two-stage deferral (ABS at lag PIPE, CLAMP+matmuls+drains at lag PIPE2)
  keeps cross-engine producers off in-order queue heads, and DMA issue
  alternates the SP/ACT queues.

  TimelineSim: 106.9us/core (v7 baseline: 308.7us).  Rel L2 err 1.44e-2,
  dominated by the deterministic fp8-e3m4 rhs/lhsT quantization.
"""

import os
import sys
from collections import deque

import numpy as np

sys.path.insert(0, "/opt/trn_rl_repo")

import ml_dtypes

import concourse.bass as bass
import concourse.bacc as bacc
import concourse.mybir as mybir
import concourse.tile as tile
from concourse.bass_utils import run_bass_kernel_spmd

H, W, BINS = 480, 640, 10
NSEG = BINS - 1          # 9 time segments (t*=9 folds into seg 8 with f=1)
P = 128
WY = 32                  # y-window width
WX = 32                  # x-window width
NQ32 = (H + WY - 1) // WY   # 15
NR32 = (W + WX - 1) // WX   # 20
NQ = 4                   # 128-tall psum stripes
NKEY = NSEG * NQ32 * NR32   # 2700 buckets
N_CORES = 8
G = int(os.environ.get("EVS8_G", "63"))        # tiles per batched group
CHUNK = int(os.environ.get("EVS8_CHUNK", "32"))  # rhs tiles per DMA chunk

# static engine mixes (percent)
SCAT = int(os.environ.get("EVS8_SCAT", "72"))   # share of groups built by
#                                         gpsimd local_scatter
HOSTM = int(os.environ.get("EVS8_HOSTM", "28"))  # share with host-shipped
#                                         m tiles (zero engine work; rest
#                                         of the groups use the chain)
SUB_DVE = int(os.environ.get("EVS8_SUB_DVE", "100"))     # rest -> Pool
ABS_ACT = int(os.environ.get("EVS8_ABS_ACT", "100"))
ABS_DVE = int(os.environ.get("EVS8_ABS_DVE", "0"))      # rest -> Pool
DRAIN_CA = int(os.environ.get("EVS8_DRAIN_CA", "704"))  # ACT cols of 1280
PIPE = int(os.environ.get("EVS8_PIPE", "3"))
CHB = int(os.environ.get("EVS8_CHB", "6"))
TAPER = int(os.environ.get("EVS8_TAPER", "126"))

F32 = mybir.dt.float32
F16 = mybir.dt.float16
F8 = mybir.dt.float8e3

_prog_cache: dict = {}


def _host_prep(ev):
    """Bucket one batch's events; returns (counts[NKEY], pack data)."""
    if ev.shape[0] == 0:
        ev = np.array([[0.0, 0.0, 0.25, 0.0, 0.0],
                       [0.0, 0.0, 0.75, 0.0, 0.0]], np.float32)
    x = ev[:, 0].astype(np.float64)
    y = ev[:, 1].astype(np.float64)
    t = ev[:, 2].astype(np.float64)
    p = ev[:, 3].astype(np.float32)
    t0 = t[0]
    tN = t[-1]
    denom = tN - t0
    if denom > 0:
        tp = (BINS - 1) * np.clip((t - t0) / denom, 0.0, 1.0)
    else:
        tp = np.zeros_like(t)
    s = np.clip(np.floor(tp).astype(np.int32), 0, NSEG - 1)
    f = (tp - s).astype(np.float64)

    iy = np.floor(y).astype(np.int64)
    icy = np.ceil(y).astype(np.int64)
    ix = np.floor(x).astype(np.int64)
    icx = np.ceil(x).astype(np.int64)
    qf, qc = iy // WY, icy // WY
    rf, rc = ix // WX, icx // WX
    n = len(x)
    idx0 = np.arange(n, dtype=np.int64)

    ys = qf != qc
    xs = rf != rc
    both = ys & xs
    inst_idx = np.concatenate([idx0, idx0[ys], idx0[xs], idx0[both]])
    inst_q = np.concatenate([qf, qc[ys], qf[xs], qc[both]])
    inst_r = np.concatenate([rf, rf[ys], rc[xs], rc[both]])
    key = (s[inst_idx] * NQ32 + inst_q) * NR32 + inst_r
    counts = np.bincount(key, minlength=NKEY)
    return counts, (x, y, f, p, ix, inst_idx, inst_q, inst_r, key)


def _pack_core(pack, tiles_per_key, T_tot):
    x, y, f, p, ix, inst_idx, inst_q, inst_r, key = pack
    col0 = np.zeros(NKEY + 1, np.int64)
    col0[1:] = np.cumsum(tiles_per_key)
    order = np.argsort(key, kind="stable")
    skey = key[order]
    sidx = inst_idx[order]
    sq = inst_q[order]
    sr = inst_r[order]
    group_start = np.searchsorted(skey, np.arange(NKEY))
    rank = np.arange(len(skey)) - group_start[skey]
    slot = col0[skey] * P + rank
    part = (slot % P).astype(np.int64)
    col = (slot // P).astype(np.int64)

    yrel = (y[sidx] - WY * sq).astype(np.float64)

    # schedule-derived per-tile info: group base + rhs sign (+hat for scat
    # groups, -hat for chain groups)
    (groups, route_seq, _, _, chain_base, scat_base,
     hostm_base) = _schedule(T_tot)
    g0s = np.array([g[0] for g in groups], np.int64)
    tile_g = np.searchsorted(g0s, np.arange(T_tot), side="right") - 1
    rcode = np.array([{"scat": 0, "hostm": 1, "chain": 2}[route_seq[g]]
                      for g in range(len(groups))])
    tile_sign = np.where(rcode[tile_g] == 2, -1.0, 1.0)   # [T_tot]
    tile_base = g0s[tile_g]            # group start tile of each tile
    cbase = np.array(chain_base, np.int64)
    sbase = np.array(scat_base, np.int64)
    hbase = np.array(hostm_base, np.int64)
    # compact per-tile destination columns
    last_n = groups[-1][1] - groups[-1][0]
    lr = rcode[-1]
    n_scat = max(1, int(sbase[-1] + (last_n if lr == 0 else 0)))
    n_hostm = max(1, int(hbase[-1] + (last_n if lr == 1 else 0)))
    n_chain = max(1, int(cbase[-1] + (last_n if lr == 2 else 0)))
    tile_local = np.arange(T_tot) - tile_base
    tile_ccol = cbase[tile_g] + tile_local      # chain-compact column
    tile_scol = sbase[tile_g] + tile_local      # scat-compact column
    tile_hcol = hbase[tile_g] + tile_local      # hostm-compact column

    Yc = np.zeros((P, n_chain), np.float32)
    chain_slots = rcode[tile_g[col]] == 2
    Yc[part[chain_slots], tile_ccol[col[chain_slots]]] = \
        yrel[chain_slots].astype(np.float32)

    # y taps; consumed by the scat groups (as scatter data/idx) and the
    # hostm groups (as a dense prebuilt +hat lhsT)
    cy0 = np.floor(yrel).astype(np.int64)
    fy = yrel - cy0
    YD = np.zeros((P, 2 * n_scat), np.float16)
    YI = np.full((P, 2 * n_scat), -1, np.int16)
    MH = np.zeros((P, WY * n_hostm), ml_dtypes.float8_e3m4)
    local = (col - tile_base[col]) * WY
    scol2 = 2 * tile_scol[col]
    ss = rcode[tile_g[col]] == 0
    hh = rcode[tile_g[col]] == 1
    hcol = tile_hcol[col] * WY
    for k, (c, h) in enumerate(((cy0, 1.0 - fy), (cy0 + 1, fy))):
        ok = ss & (c >= 0) & (c < WY)
        YD[part[ss], scol2[ss] + k] = h[ss].astype(np.float16)
        YI[part[ok], scol2[ok] + k] = (local[ok] + c[ok]).astype(np.int16)
        okh = hh & (c >= 0) & (c < WY)
        MH[part[okh], hcol[okh] + c[okh]] = \
            h[okh].astype(ml_dtypes.float8_e3m4)

    # rhs: 64 interleaved cols (2*cx + b) = sign * 8 * p * w_b * hx[cx]
    xi = x[sidx]
    fi = f[sidx]
    pi = p[sidx].astype(np.float64)
    ixi = ix[sidx]
    s8 = 8.0 * tile_sign[col] * pi
    cf = ixi - WX * sr           # floor-tap col (may be -1 for ceil dups)
    cc = cf + 1                  # ceil-tap col (may be 32 for floor side)
    hx1 = xi - ixi               # ceil-tap weight
    hx0 = 1.0 - hx1
    w0 = s8 * (1.0 - fi)
    w1 = s8 * fi
    RHS = np.zeros((P, T_tot, 2 * WX), np.float32)
    for valid, c, h in ((cf >= 0, cf, hx0), (cc < WX, cc, hx1)):
        for b, wv in ((0, w0), (1, w1)):
            RHS[part[valid], col[valid], 2 * c[valid] + b] = \
                (h * wv)[valid].astype(np.float32)
    RHS8 = RHS.reshape(P, T_tot * 2 * WX).astype(ml_dtypes.float8_e3m4)
    return {"ev_y": Yc, "ev_rhs": RHS8, "ev_yd": YD,
            "ev_yi": YI, "ev_mh": MH}


def _mix_seq(fracs, n):
    """Maximally-even interleave (error diffusion) of engine choices."""
    tot = max(1, sum(fracs.values()))
    fr = {k: v / tot for k, v in fracs.items()}
    cnt = {k: 0 for k in fr}
    seq = []
    for i in range(n):
        pick = max(fr, key=lambda k: fr[k] * (i + 1) - cnt[k])
        cnt[pick] += 1
        seq.append(pick)
    return seq


def _schedule(T_tot):
    """Group slices + per-group route assignment, shared host/device.

    Returns (groups, route_seq, sub_seq, abs_seq, chain_base, scat_base):
    the *_base lists give each group's start offset in the compact
    chain-y / scat-data arrays (in tiles)."""
    bounds = list(range(0, max(0, T_tot - TAPER), G))
    g_small = max(8, G // 4)
    bounds += list(range(max(0, T_tot - TAPER), T_tot, g_small))
    bounds.append(T_tot)
    groups = [(a, b) for a, b in zip(bounds[:-1], bounds[1:]) if b > a]
    n = len(groups)
    route_seq = _mix_seq({"scat": SCAT, "hostm": HOSTM,
                          "chain": max(0, 100 - SCAT - HOSTM)}, n)
    nchain = sum(1 for r in route_seq if r == "chain")
    sub_seq = _mix_seq({"dve": SUB_DVE, "pool": 100 - SUB_DVE}, nchain)
    abs_seq = _mix_seq({"act": ABS_ACT, "dve": ABS_DVE,
                        "pool": max(0, 100 - ABS_ACT - ABS_DVE)}, nchain)
    chain_base, scat_base, hostm_base = [], [], []
    cb = sb = hb = 0
    for g, (a, b) in enumerate(groups):
        chain_base.append(cb)
        scat_base.append(sb)
        hostm_base.append(hb)
        if route_seq[g] == "scat":
            sb += b - a
        elif route_seq[g] == "hostm":
            hb += b - a
        else:
            cb += b - a
    return (groups, route_seq, sub_seq, abs_seq, chain_base, scat_base,
            hostm_base)


def _build_program(tiles_per_key, T_tot):
    nc = bacc.Bacc("TRN2", debug=False)
    (groups, route_seq, sub_seq, abs_seq, chain_base, scat_base,
     hostm_base) = _schedule(T_tot)
    ngroups = len(groups)
    last_n = groups[-1][1] - groups[-1][0]
    lr = route_seq[-1]
    n_scat = max(1, scat_base[-1] + (last_n if lr == "scat" else 0))
    n_hostm = max(1, hostm_base[-1] + (last_n if lr == "hostm" else 0))
    n_chain = max(1, chain_base[-1] + (last_n if lr == "chain" else 0))
    y_d = nc.dram_tensor("ev_y", [P, n_chain], F32, kind="ExternalInput")
    yd_d = nc.dram_tensor("ev_yd", [P, 2 * n_scat], F16,
                          kind="ExternalInput")
    yi_d = nc.dram_tensor("ev_yi", [P, 2 * n_scat], mybir.dt.int16,
                          kind="ExternalInput")
    rhs_d = nc.dram_tensor("ev_rhs", [P, T_tot * 2 * WX], F8,
                           kind="ExternalInput")
    mh_d = nc.dram_tensor("ev_mh", [P, WY * n_hostm], F8,
                          kind="ExternalInput")
    out_d = nc.dram_tensor("outv", [BINS, NQ, P, W], F16,
                           kind="ExternalOutput")

    col0 = np.zeros(NKEY + 1, np.int64)
    col0[1:] = np.cumsum(tiles_per_key)

    Alu = mybir.AluOpType
    Act = mybir.ActivationFunctionType

    # global tile stream: (c, sq_idx, ghat, r, first, last)
    tiles = []
    sq_last_tile = {}
    for si in range(NSEG):
        for qi in range(NQ):
            sqi = si * NQ + qi
            for q32 in range(4 * qi, min(4 * qi + 4, NQ32)):
                for r in range(NR32):
                    k = (si * NQ32 + q32) * NR32 + r
                    ntile = int(tiles_per_key[k])
                    cbase = int(col0[k])
                    for j in range(ntile):
                        tiles.append((cbase + j, sqi, q32 - 4 * qi, r,
                                      j == 0, j == ntile - 1))
                        sq_last_tile[sqi] = len(tiles) - 1
    assert len(tiles) == T_tot


    with tile.TileContext(nc) as tc:
        with (
            tc.tile_pool(name="persist", bufs=1) as persist,
            tc.tile_pool(name="psum", bufs=int(os.environ.get("EVS8_PSB", "2")), space="PSUM") as psump,
            tc.tile_pool(name="chunk", bufs=CHB) as chp,
            tc.tile_pool(name="dg", bufs=int(os.environ.get("EVS8_DB", "8"))) as dp,
            tc.tile_pool(name="zg", bufs=int(os.environ.get("EVS8_DB", "8"))) as zp,
            tc.tile_pool(name="mg", bufs=int(os.environ.get("EVS8_MB", "8"))) as mp,
            tc.tile_pool(name="mh", bufs=4) as mhp,
        ):
            dmaq = [nc.sync, nc.scalar]
            dmaqi = [0]

            def dma_rr(**kw):
                eng = dmaq[dmaqi[0] % len(dmaq)]
                dmaqi[0] += 1
                eng.dma_start(**kw)

            # --- y values (small first chunk so group 0 starts ASAP)
            yt = persist.tile([P, n_chain], F32, tag="yt")
            ydt = persist.tile([P, 2 * n_scat], F16, tag="ydt")
            yit = persist.tile([P, 2 * n_scat], mybir.dt.int16, tag="yit")
            for tot, dst, src_d, m in ((n_chain, yt, y_d, 1),
                                       (n_scat, ydt, yd_d, 2),
                                       (n_scat, yit, yi_d, 2)):
                cuts = [0, tot // 16, tot // 4, tot // 2,
                        3 * tot // 4, tot]
                for y0, y1 in zip(cuts[:-1], cuts[1:]):
                    if y1 > y0:
                        dma_rr(out=dst[:, m * y0:m * y1],
                               in_=src_d[:, m * y0:m * y1])

            # --- V ring: 3 generations x NQ q-blocks of one output plane
            VR = 3
            vring = persist.tile([P, VR * NQ * W], F16, tag="vring")

            # --- constants: io = iota cols 0..31 (f16), ACT table warm
            ioi = persist.tile([P, WY], mybir.dt.int32, tag="ioi")
            nc.gpsimd.iota(ioi[:], pattern=[[1, WY]], base=0,
                           channel_multiplier=0)
            io16 = persist.tile([P, WY], F16, tag="io16")
            nc.vector.tensor_copy(io16[:], ioi[:])
            warm = persist.tile([1, 1], F16, tag="warm")
            nc.vector.memset(warm[:], 0.0)
            nc.scalar.activation(warm[:], warm[:], Act.Abs)

            # --- rhs chunk streaming
            chunk_tiles: dict = {}

            def get_chunk(ch):
                if ch not in chunk_tiles:
                    t = chp.tile([P, CHUNK * 2 * WX], F8, tag="ch")
                    lo = ch * CHUNK * 2 * WX
                    hi = min((ch + 1) * CHUNK * 2 * WX, T_tot * 2 * WX)
                    dma_rr(out=t[:, 0:hi - lo], in_=rhs_d[:, lo:hi])
                    chunk_tiles[ch] = t
                return chunk_tiles[ch]

            for _pc in range(4):
                get_chunk(_pc)

            psum_tiles: dict = {}
            ph_q = deque()
            ph_q2 = deque()
            PIPE2 = int(os.environ.get("EVS8_PIPE2", str(2 * PIPE + 4)))

            def flush(keep):
                while len(ph_q) > keep:
                    ph_q.popleft()()

            def flush2(keep):
                while len(ph_q2) > keep:
                    ph_q2.popleft()()

            def get_psum(sqi):
                if sqi not in psum_tiles:
                    psum_tiles[sqi] = psump.tile([P, 2 * W], F32, tag="ps",
                                                 name=f"ps{sqi % int(os.environ.get('EVS8_PSB', '2'))}",
                                                 uniquify=True)
                return psum_tiles[sqi]

            def emit_mms(gtiles, mg):
                for j, (c, sqi, gh, r, first, last) in enumerate(gtiles):
                    ch, lo = divmod(c, CHUNK)
                    rhs_t = get_chunk(ch)
                    ps = get_psum(sqi)
                    nc.tensor.matmul(
                        ps[WY * gh:WY * (gh + 1),
                           2 * WX * r:2 * WX * (r + 1)],
                        lhsT=mg[:, j * WY:(j + 1) * WY],
                        rhs=rhs_t[:, lo * 2 * WX:(lo + 1) * 2 * WX],
                        start=first, stop=last,
                        tile_position=(0, WY * gh))

            def make_abs(ci, gw, dg, cell):
                def phA():
                    ae = abs_seq[ci]
                    zg = zp.tile([P, G * WY], F16, tag="zg")
                    if ae == "act":
                        nc.scalar.activation(zg[:, 0:gw], dg[:, 0:gw],
                                             Act.Abs)
                    elif ae == "dve":
                        nc.vector.scalar_tensor_tensor(
                            zg[:, 0:gw], dg[:, 0:gw], -1.0, dg[:, 0:gw],
                            op0=Alu.mult, op1=Alu.max)
                    else:
                        nc.gpsimd.scalar_tensor_tensor(
                            zg[:, 0:gw], dg[:, 0:gw], -1.0, dg[:, 0:gw],
                            op0=Alu.mult, op1=Alu.max)
                    cell["zg"] = zg

                return phA

            def make_mms(route, gtiles, gw, cell):
                def phB():
                    if route in ("scat", "hostm"):
                        mg = cell["mg"]
                    else:
                        zg = cell["zg"]
                        mg = mp.tile([P, G * WY], F16, tag="mg")
                        nc.vector.tensor_scalar(mg[:, 0:gw], zg[:, 0:gw],
                                                1.0, 0.0,
                                                op0=Alu.subtract,
                                                op1=Alu.min)
                    emit_mms(gtiles, mg)

                return phB

            def vslot(plane, qi):
                base = ((plane % 3) * NQ + qi) * W
                return vring[:, base:base + W]

            def make_drain_pieces(sqi):
                """Merge the two bin half-planes into the V ring (copy +
                add, same column count as two copies) and DMA finalized
                planes; halves output bytes vs shipping both halves."""
                si, qi = divmod(sqi, NQ)
                rows = min(P, H - P * qi)

                def pc_act():
                    ps = psum_tiles[sqi]
                    pv = ps[0:rows, :].rearrange("p (x b) -> p x b", b=2)
                    # open plane si+1 with segment si's b1 half
                    nc.scalar.activation(vslot(si + 1, qi)[0:rows, :],
                                         pv[:, :, 1], Act.Copy)

                def pc_dve():
                    ps = psum_tiles.pop(sqi)
                    pv = ps[0:rows, :].rearrange("p (x b) -> p x b", b=2)
                    vv = vslot(si, qi)[0:rows, :]
                    if si == 0:
                        nc.vector.tensor_copy(vv, pv[:, :, 0])
                    else:
                        nc.vector.scalar_tensor_tensor(
                            vv, pv[:, :, 0], 0.0, vv,
                            op0=Alu.add, op1=Alu.add)
                    # plane si's q-block is final now
                    dma_rr(out=out_d[si, qi, 0:rows, :], in_=vv)
                    if si == NSEG - 1:
                        dma_rr(out=out_d[NSEG, qi, 0:rows, :],
                               in_=vslot(NSEG, qi)[0:rows, :])

                return [pc_act, pc_dve]

            last_to_sq = {v: k for k, v in sq_last_tile.items()}
            pending_pieces = deque()
            ci = 0  # chain-group ordinal
            for gi, (g0, g1) in enumerate(groups):
                gtiles = tiles[g0:g1]
                gn = len(gtiles)
                c0 = gtiles[0][0]
                # touch psums in stream order so pool cycling stays sane
                for tl in gtiles:
                    get_psum(tl[1])
                cell = {}
                if route_seq[gi] == "hostm":
                    # lhsT shipped prebuilt from the host: one DMA, no
                    # engine work at all
                    hb = hostm_base[gi]
                    mg = mhp.tile([P, G * WY], F8, tag="mh", name="mh")
                    dma_rr(out=mg[:, 0:gn * WY],
                           in_=mh_d[:, WY * hb:WY * (hb + gn)])
                    cell["mg"] = mg
                    ph_q.append(lambda: None)
                elif route_seq[gi] == "scat":
                    # whole m tile built by one gpsimd scatter of the
                    # per-event (1-fy, fy) taps
                    sb = scat_base[gi]
                    mg = mp.tile([P, G * WY], F16, tag="mgs", name="mgs")
                    nc.gpsimd.local_scatter(
                        mg[:, 0:gn * WY], ydt[:, 2 * sb:2 * (sb + gn)],
                        yit[:, 2 * sb:2 * (sb + gn)], channels=P,
                        num_elems=gn * WY, num_idxs=2 * gn)
                    cell["mg"] = mg
                    ph_q.append(lambda: None)
                else:
                    dg = dp.tile([P, G * WY], F16, tag="dgt")
                    iob = io16[:].rearrange("p (o c) -> p o c", o=1) \
                        .to_broadcast([P, gn, WY])
                    cb = chain_base[gi]
                    yb = yt[:, cb:cb + gn] \
                        .rearrange("p (g o) -> p g o", o=1) \
                        .to_broadcast([P, gn, WY])
                    dgv = dg[:, 0:gn * WY].rearrange("p (g c) -> p g c",
                                                     g=gn)
                    if sub_seq[ci] == "dve":
                        nc.vector.tensor_tensor(dgv, iob, yb,
                                                op=Alu.subtract)
                    else:
                        nc.gpsimd.tensor_tensor(dgv, iob, yb,
                                                op=Alu.subtract)
                    ph_q.append(make_abs(ci, gn * WY, dg, cell))
                    ci += 1
                ph_q2.append(make_mms(route_seq[gi], gtiles, gn * WY, cell))
                # drains queue behind the completing group's matmuls
                for ti in range(g0, g0 + gn):
                    if ti in last_to_sq:
                        for pc in make_drain_pieces(last_to_sq[ti]):
                            ph_q2.append(pc)
                flush(PIPE)
                flush2(PIPE2)
            flush(0)
            flush2(0)
    nc.finalize()
    return nc


def kernel(events, lengths):
    events = np.ascontiguousarray(events, dtype=np.float32)
    lengths = np.asarray(lengths)
    B = int(lengths.shape[0])
    offs = np.zeros(B + 1, np.int64)
    offs[1:] = np.cumsum(lengths)

    packs = []
    counts = np.zeros((B, NKEY), np.int64)
    for bi in range(B):
        c, pk = _host_prep(events[offs[bi]:offs[bi + 1]])
        counts[bi] = c
        packs.append(pk)

    tiles_per_key = np.maximum(1, -(-counts.max(axis=0) // P)).astype(np.int64)
    T_tot = int(tiles_per_key.sum())

    key = (tuple(tiles_per_key.tolist()), T_tot, G, CHUNK, SCAT, TAPER,
           HOSTM, SUB_DVE, ABS_ACT, ABS_DVE, DRAIN_CA, PIPE,
           os.environ.get("EVS8_PIPE2", ""),
           os.environ.get("EVS8_PSB", "2"),
           os.environ.get("EVS8_DB", "8"), os.environ.get("EVS8_MB", "8"))
    if key not in _prog_cache:
        _prog_cache[key] = _build_program(tiles_per_key, T_tot)
    nc = _prog_cache[key]

    in_maps = [_pack_core(pk, tiles_per_key, T_tot) for pk in packs]
    trace = bool(int(os.environ.get("EVS_TRACE", "0")))
    res = run_bass_kernel_spmd(nc, in_maps, core_ids=list(range(B)),
                               trace=trace)
    global last_results
    last_results = res

    out = np.zeros((B, BINS, H, W), np.float32)
    for bi in range(B):
        ov = np.asarray(res.results[bi]["outv"]).astype(np.float32)
        out[bi] = ov.reshape(BINS, NQ * P, W)[:, :H] * (1.0 / 8.0)
    return out


last_results = None


if __name__ == "__main__":
    # tiny smoke test with synthetic events
    rng = np.random.default_rng(0)
    B0, NP0 = 8, 2000
    N0 = B0 * NP0
    x = rng.uniform(0, W - 1, N0).astype(np.float32)
    y = rng.uniform(0, H - 1, N0).astype(np.float32)
    t = np.sort(rng.uniform(0, 1, (B0, NP0)).astype(np.float32), axis=1).ravel()
    p = (2.0 * rng.integers(0, 2, N0) - 1).astype(np.float32)
    b = np.repeat(np.arange(B0), NP0).astype(np.float32)
    ev = np.stack([x, y, t, p, b], axis=1)
    ln = np.full(B0, NP0, np.int32)
    out = kernel(ev, ln)
    # numpy reference
    ref = np.zeros((B0, BINS, H, W), np.float64)
    for bi in range(B0):
        sl = slice(bi * NP0, (bi + 1) * NP0)
        xx, yy, tt2, pp = x[sl], y[sl], t[sl], p[sl]
        t0, tN = tt2[0], tt2[-1]
        ts = (BINS - 1) * np.clip((tt2 - t0) / (tN - t0), 0, 1)
        import itertools
        for xr_f, yr_f, br_f in itertools.product([np.floor, np.ceil], repeat=3):
            xr, yr, br = xr_f(xx), yr_f(yy), br_f(ts)
            valid = (((xr != xx) | (xr_f is np.floor))
                     & ((yr != yy) | (yr_f is np.floor))
                     & ((br != ts) | (br_f is np.floor))
                     & (xr < W) & (yr < H) & (br < BINS))
            kb = lambda a_: np.maximum(0, 1 - np.abs(a_))
            val = np.where(valid, pp * kb(xr - xx) * kb(yr - yy) * kb(br - ts), 0)
            np.add.at(ref[bi].ravel(),
                      np.where(valid, (xr + yr * W + br * H * W).astype(np.int64), 0),
                      val)
    num = np.linalg.norm((out - ref).ravel())
    den = np.linalg.norm(ref.ravel())
    print("smoke rel l2 err:", num / max(den, 1e-30))
    print("smoke max abs err:", np.abs(out - ref).max())


# revision 51
# speedup vs baseline: 1.0335x; 1.0005x over previous
"""EventVolumeSurface trilinear voxel-grid kernel for Trainium2 (Bass/Tile).

v10 strategy (data-parallel over batch, 1 batch -> 1 NeuronCore):
  Events are bucketed by (time-segment s in [0,9), y-window q32 = iy>>5 in
  [0,15), x-window r32 = ix>>5 in [0,20)) with straddle duplication at the
  32-boundaries.  Tiles of 128 event slots stream in bucket order, batched
  into groups of G=63.

  Host ships, per 128-event tile slot:
    - rhs [128, T*64] fp8 e3m4: 64 interleaved columns (2*cx + b) holding
      sign * 8 * p * w_b * hx[cx]; hx = 2-tap x hat, w_0 = 1-f, w_1 = f
      (time-bin weights); x8 keeps values in e3m4's normal range (host
      divides by 8 at unshard); sign matches the group route below.
    - scat groups (75%): per-slot y taps (1-fy, fy) f16 + int16
      group-local scatter indices (-1 = skip, handles window straddle).
    - hostm groups (28%): the +hat lhsT tile prebuilt on the host, in
      fp8 e3m4 (unscaled hat values, so no psum-scale mixing).
    - chain groups (0% by default, kept as a tuning fallback): yhat f32.

  Device m-tile build (columns cost engine time; partitions are free):
    scat:  ONE gpsimd local_scatter builds the whole [128, 63*32] +hat
           lhsT group on Pool (~47ns/tile, replaces sub/abs/min chains)
    hostm: one DMA, zero engine work (spends spare DMA bandwidth to
           relieve Pool; the 75/25 split equalizes Pool vs DMA)
    chain: DVE broadcast-subtract -> ACT Abs -> DVE fused min (= -hat)
    MM: psum[32*g:+32, 64*r:+64] += m_j^T @ rhs_j  (f16 x fp8e3, 64 out
        cols ~36ns; tile_position puts the 32 out rows at partition 32*g)
  PSUM is one [128, 1280] f32 tile per (s, q128) stripe with interleaved
  (x, bin) columns (bin-strided matmul out APs are broken on HW).  Drains
  merge the two bin half-planes on-device into a 3-deep V ring (ACT copy
  of the b1 half opens plane s+1, DVE add of the b0 half finalizes plane
  s), so the output DMA ships each plane once: 6.1 MB f16 total.  A
  two-stage deferral (ABS at lag PIPE, CLAMP+matmuls+drains at lag PIPE2)
  keeps cross-engine producers off in-order queue heads, and DMA issue
  alternates the SP/ACT queues.

  TimelineSim: 106.9us/core (v7 baseline: 308.7us).  Rel L2 err 1.44e-2,
  dominated by the deterministic fp8-e3m4 rhs/lhsT quantization.
"""

import os
import sys
from collections import deque

import numpy as np

sys.path.insert(0, "/opt/trn_rl_repo")

import ml_dtypes

import concourse.bass as bass
import concourse.bacc as bacc
import concourse.mybir as mybir
import concourse.tile as tile
from concourse.bass_utils import run_bass_kernel_spmd

H, W, BINS = 480, 640, 10
NSEG = BINS - 1          # 9 time segments (t*=9 folds into seg 8 with f=1)
P = 128
WY = 32                  # y-window width
WX = 32                  # x-window width
NQ32 = (H + WY - 1) // WY   # 15
NR32 = (W + WX - 1) // WX   # 20
NQ = 4                   # 128-tall psum stripes
NKEY = NSEG * NQ32 * NR32   # 2700 buckets
N_CORES = 8
G = int(os.environ.get("EVS8_G", "63"))        # tiles per batched group
CHUNK = int(os.environ.get("EVS8_CHUNK", "32"))  # rhs tiles per DMA chunk

# static engine mixes (percent)
SCAT = int(os.environ.get("EVS8_SCAT", "72"))   # share of groups built by
#                                         gpsimd local_scatter
HOSTM = int(os.environ.get("EVS8_HOSTM", "28"))  # share with host-shipped
#                                         m tiles (zero engine work; rest
#                                         of the groups use the chain)
SUB_DVE = int(os.environ.get("EVS8_SUB_DVE", "100"))     # rest -> Pool
ABS_ACT = int(os.environ.get("EVS8_ABS_ACT", "100"))
ABS_DVE = int(os.environ.get("EVS8_ABS_DVE", "0"))      # rest -> Pool
DRAIN_CA = int(os.environ.get("EVS8_DRAIN_CA", "704"))  # ACT cols of 1280
PIPE = int(os.environ.get("EVS8_PIPE", "3"))
CHB = int(os.environ.get("EVS8_CHB", "6"))
TAPER = int(os.environ.get("EVS8_TAPER", "126"))

F32 = mybir.dt.float32
F16 = mybir.dt.float16
F8 = mybir.dt.float8e3

_prog_cache: dict = {}


def _host_prep(ev):
    """Bucket one batch's events; returns (counts[NKEY], pack data)."""
    if ev.shape[0] == 0:
        ev = np.array([[0.0, 0.0, 0.25, 0.0, 0.0],
                       [0.0, 0.0, 0.75, 0.0, 0.0]], np.float32)
    x = ev[:, 0].astype(np.float64)
    y = ev[:, 1].astype(np.float64)
    t = ev[:, 2].astype(np.float64)
    p = ev[:, 3].astype(np.float32)
    t0 = t[0]
    tN = t[-1]
    denom = tN - t0
    if denom > 0:
        tp = (BINS - 1) * np.clip((t - t0) / denom, 0.0, 1.0)
    else:
        tp = np.zeros_like(t)
    s = np.clip(np.floor(tp).astype(np.int32), 0, NSEG - 1)
    f = (tp - s).astype(np.float64)

    iy = np.floor(y).astype(np.int64)
    icy = np.ceil(y).astype(np.int64)
    ix = np.floor(x).astype(np.int64)
    icx = np.ceil(x).astype(np.int64)
    qf, qc = iy // WY, icy // WY
    rf, rc = ix // WX, icx // WX
    n = len(x)
    idx0 = np.arange(n, dtype=np.int64)

    ys = qf != qc
    xs = rf != rc
    both = ys & xs
    inst_idx = np.concatenate([idx0, idx0[ys], idx0[xs], idx0[both]])
    inst_q = np.concatenate([qf, qc[ys], qf[xs], qc[both]])
    inst_r = np.concatenate([rf, rf[ys], rc[xs], rc[both]])
    key = (s[inst_idx] * NQ32 + inst_q) * NR32 + inst_r
    counts = np.bincount(key, minlength=NKEY)
    return counts, (x, y, f, p, ix, inst_idx, inst_q, inst_r, key)


def _pack_core(pack, tiles_per_key, T_tot):
    x, y, f, p, ix, inst_idx, inst_q, inst_r, key = pack
    col0 = np.zeros(NKEY + 1, np.int64)
    col0[1:] = np.cumsum(tiles_per_key)
    order = np.argsort(key, kind="stable")
    skey = key[order]
    sidx = inst_idx[order]
    sq = inst_q[order]
    sr = inst_r[order]
    group_start = np.searchsorted(skey, np.arange(NKEY))
    rank = np.arange(len(skey)) - group_start[skey]
    slot = col0[skey] * P + rank
    part = (slot % P).astype(np.int64)
    col = (slot // P).astype(np.int64)

    yrel = (y[sidx] - WY * sq).astype(np.float64)

    # schedule-derived per-tile info: group base + rhs sign (+hat for scat
    # groups, -hat for chain groups)
    (groups, route_seq, _, _, chain_base, scat_base,
     hostm_base) = _schedule(T_tot)
    g0s = np.array([g[0] for g in groups], np.int64)
    tile_g = np.searchsorted(g0s, np.arange(T_tot), side="right") - 1
    rcode = np.array([{"scat": 0, "hostm": 1, "chain": 2}[route_seq[g]]
                      for g in range(len(groups))])
    tile_sign = np.where(rcode[tile_g] == 2, -1.0, 1.0)   # [T_tot]
    tile_base = g0s[tile_g]            # group start tile of each tile
    cbase = np.array(chain_base, np.int64)
    sbase = np.array(scat_base, np.int64)
    hbase = np.array(hostm_base, np.int64)
    # compact per-tile destination columns
    last_n = groups[-1][1] - groups[-1][0]
    lr = rcode[-1]
    n_scat = max(1, int(sbase[-1] + (last_n if lr == 0 else 0)))
    n_hostm = max(1, int(hbase[-1] + (last_n if lr == 1 else 0)))
    n_chain = max(1, int(cbase[-1] + (last_n if lr == 2 else 0)))
    tile_local = np.arange(T_tot) - tile_base
    tile_ccol = cbase[tile_g] + tile_local      # chain-compact column
    tile_scol = sbase[tile_g] + tile_local      # scat-compact column
    tile_hcol = hbase[tile_g] + tile_local      # hostm-compact column

    Yc = np.zeros((P, n_chain), np.float32)
    chain_slots = rcode[tile_g[col]] == 2
    Yc[part[chain_slots], tile_ccol[col[chain_slots]]] = \
        yrel[chain_slots].astype(np.float32)

    # y taps; consumed by the scat groups (as scatter data/idx) and the
    # hostm groups (as a dense prebuilt +hat lhsT)
    cy0 = np.floor(yrel).astype(np.int64)
    fy = yrel - cy0
    YD = np.zeros((P, 2 * n_scat), np.float16)
    YI = np.full((P, 2 * n_scat), -1, np.int16)
    MH = np.zeros((P, WY * n_hostm), ml_dtypes.float8_e3m4)
    local = (col - tile_base[col]) * WY
    scol2 = 2 * tile_scol[col]
    ss = rcode[tile_g[col]] == 0
    hh = rcode[tile_g[col]] == 1
    hcol = tile_hcol[col] * WY
    for k, (c, h) in enumerate(((cy0, 1.0 - fy), (cy0 + 1, fy))):
        ok = ss & (c >= 0) & (c < WY)
        YD[part[ss], scol2[ss] + k] = h[ss].astype(np.float16)
        YI[part[ok], scol2[ok] + k] = (local[ok] + c[ok]).astype(np.int16)
        okh = hh & (c >= 0) & (c < WY)
        MH[part[okh], hcol[okh] + c[okh]] = \
            h[okh].astype(ml_dtypes.float8_e3m4)

    # rhs: 64 interleaved cols (2*cx + b) = sign * 8 * p * w_b * hx[cx]
    xi = x[sidx]
    fi = f[sidx]
    pi = p[sidx].astype(np.float64)
    ixi = ix[sidx]
    s8 = 8.0 * tile_sign[col] * pi
    cf = ixi - WX * sr           # floor-tap col (may be -1 for ceil dups)
    cc = cf + 1                  # ceil-tap col (may be 32 for floor side)
    hx1 = xi - ixi               # ceil-tap weight
    hx0 = 1.0 - hx1
    w0 = s8 * (1.0 - fi)
    w1 = s8 * fi
    RHS = np.zeros((P, T_tot, 2 * WX), np.float32)
    for valid, c, h in ((cf >= 0, cf, hx0), (cc < WX, cc, hx1)):
        for b, wv in ((0, w0), (1, w1)):
            RHS[part[valid], col[valid], 2 * c[valid] + b] = \
                (h * wv)[valid].astype(np.float32)
    RHS8 = RHS.reshape(P, T_tot * 2 * WX).astype(ml_dtypes.float8_e3m4)
    return {"ev_y": Yc, "ev_rhs": RHS8, "ev_yd": YD,
            "ev_yi": YI, "ev_mh": MH}


def _mix_seq(fracs, n):
    """Maximally-even interleave (error diffusion) of engine choices."""
    tot = max(1, sum(fracs.values()))
    fr = {k: v / tot for k, v in fracs.items()}
    cnt = {k: 0 for k in fr}
    seq = []
    for i in range(n):
        pick = max(fr, key=lambda k: fr[k] * (i + 1) - cnt[k])
        cnt[pick] += 1
        seq.append(pick)
    return seq


def _schedule(T_tot):
    """Group slices + per-group route assignment, shared host/device.

    Returns (groups, route_seq, sub_seq, abs_seq, chain_base, scat_base):
    the *_base lists give each group's start offset in the compact
    chain-y / scat-data arrays (in tiles)."""
    bounds = list(range(0, max(0, T_tot - TAPER), G))
    g_small = int(os.environ.get("EVS8_GSMALL", "10"))
    bounds += list(range(max(0, T_tot - TAPER), T_tot, g_small))
    bounds.append(T_tot)
    groups = [(a, b) for a, b in zip(bounds[:-1], bounds[1:]) if b > a]
    n = len(groups)
    route_seq = _mix_seq({"scat": SCAT, "hostm": HOSTM,
                          "chain": max(0, 100 - SCAT - HOSTM)}, n)
    nchain = sum(1 for r in route_seq if r == "chain")
    sub_seq = _mix_seq({"dve": SUB_DVE, "pool": 100 - SUB_DVE}, nchain)
    abs_seq = _mix_seq({"act": ABS_ACT, "dve": ABS_DVE,
                        "pool": max(0, 100 - ABS_ACT - ABS_DVE)}, nchain)
    chain_base, scat_base, hostm_base = [], [], []
    cb = sb = hb = 0
    for g, (a, b) in enumerate(groups):
        chain_base.append(cb)
        scat_base.append(sb)
        hostm_base.append(hb)
        if route_seq[g] == "scat":
            sb += b - a
        elif route_seq[g] == "hostm":
            hb += b - a
        else:
            cb += b - a
    return (groups, route_seq, sub_seq, abs_seq, chain_base, scat_base,
            hostm_base)


def _build_program(tiles_per_key, T_tot):
    nc = bacc.Bacc("TRN2", debug=False)
    (groups, route_seq, sub_seq, abs_seq, chain_base, scat_base,
     hostm_base) = _schedule(T_tot)
    ngroups = len(groups)
    last_n = groups[-1][1] - groups[-1][0]
    lr = route_seq[-1]
    n_scat = max(1, scat_base[-1] + (last_n if lr == "scat" else 0))
    n_hostm = max(1, hostm_base[-1] + (last_n if lr == "hostm" else 0))
    n_chain = max(1, chain_base[-1] + (last_n if lr == "chain" else 0))
    y_d = nc.dram_tensor("ev_y", [P, n_chain], F32, kind="ExternalInput")
    yd_d = nc.dram_tensor("ev_yd", [P, 2 * n_scat], F16,
                          kind="ExternalInput")
    yi_d = nc.dram_tensor("ev_yi", [P, 2 * n_scat], mybir.dt.int16,
                          kind="ExternalInput")
    rhs_d = nc.dram_tensor("ev_rhs", [P, T_tot * 2 * WX], F8,
                           kind="ExternalInput")
    mh_d = nc.dram_tensor("ev_mh", [P, WY * n_hostm], F8,
                          kind="ExternalInput")
    out_d = nc.dram_tensor("outv", [BINS, NQ, P, W], F16,
                           kind="ExternalOutput")

    col0 = np.zeros(NKEY + 1, np.int64)
    col0[1:] = np.cumsum(tiles_per_key)

    Alu = mybir.AluOpType
    Act = mybir.ActivationFunctionType

    # global tile stream: (c, sq_idx, ghat, r, first, last)
    tiles = []
    sq_last_tile = {}
    for si in range(NSEG):
        for qi in range(NQ):
            sqi = si * NQ + qi
            for q32 in range(4 * qi, min(4 * qi + 4, NQ32)):
                for r in range(NR32):
                    k = (si * NQ32 + q32) * NR32 + r
                    ntile = int(tiles_per_key[k])
                    cbase = int(col0[k])
                    for j in range(ntile):
                        tiles.append((cbase + j, sqi, q32 - 4 * qi, r,
                                      j == 0, j == ntile - 1))
                        sq_last_tile[sqi] = len(tiles) - 1
    assert len(tiles) == T_tot


    with tile.TileContext(nc) as tc:
        with (
            tc.tile_pool(name="persist", bufs=1) as persist,
            tc.tile_pool(name="psum", bufs=int(os.environ.get("EVS8_PSB", "2")), space="PSUM") as psump,
            tc.tile_pool(name="chunk", bufs=CHB) as chp,
            tc.tile_pool(name="dg", bufs=int(os.environ.get("EVS8_DB", "8"))) as dp,
            tc.tile_pool(name="zg", bufs=int(os.environ.get("EVS8_DB", "8"))) as zp,
            tc.tile_pool(name="mg", bufs=int(os.environ.get("EVS8_MB", "8"))) as mp,
            tc.tile_pool(name="mh", bufs=4) as mhp,
        ):
            dmaq = [nc.sync, nc.scalar]
            dmaqi = [0]

            def dma_rr(**kw):
                eng = dmaq[dmaqi[0] % len(dmaq)]
                dmaqi[0] += 1
                eng.dma_start(**kw)

            # --- y values (small first chunk so group 0 starts ASAP)
            yt = persist.tile([P, n_chain], F32, tag="yt")
            ydt = persist.tile([P, 2 * n_scat], F16, tag="ydt")
            yit = persist.tile([P, 2 * n_scat], mybir.dt.int16, tag="yit")
            for tot, dst, src_d, m in ((n_chain, yt, y_d, 1),
                                       (n_scat, ydt, yd_d, 2),
                                       (n_scat, yit, yi_d, 2)):
                cuts = [0, tot // 16, tot // 4, tot // 2,
                        3 * tot // 4, tot]
                for y0, y1 in zip(cuts[:-1], cuts[1:]):
                    if y1 > y0:
                        dma_rr(out=dst[:, m * y0:m * y1],
                               in_=src_d[:, m * y0:m * y1])

            # --- V ring: 3 generations x NQ q-blocks of one output plane
            VR = 3
            vring = persist.tile([P, VR * NQ * W], F16, tag="vring")

            # --- constants: io = iota cols 0..31 (f16), ACT table warm
            ioi = persist.tile([P, WY], mybir.dt.int32, tag="ioi")
            nc.gpsimd.iota(ioi[:], pattern=[[1, WY]], base=0,
                           channel_multiplier=0)
            io16 = persist.tile([P, WY], F16, tag="io16")
            nc.vector.tensor_copy(io16[:], ioi[:])
            warm = persist.tile([1, 1], F16, tag="warm")
            nc.vector.memset(warm[:], 0.0)
            nc.scalar.activation(warm[:], warm[:], Act.Abs)

            # --- rhs chunk streaming
            chunk_tiles: dict = {}

            def get_chunk(ch):
                if ch not in chunk_tiles:
                    t = chp.tile([P, CHUNK * 2 * WX], F8, tag="ch")
                    lo = ch * CHUNK * 2 * WX
                    hi = min((ch + 1) * CHUNK * 2 * WX, T_tot * 2 * WX)
                    dma_rr(out=t[:, 0:hi - lo], in_=rhs_d[:, lo:hi])
                    chunk_tiles[ch] = t
                return chunk_tiles[ch]

            for _pc in range(4):
                get_chunk(_pc)

            psum_tiles: dict = {}
            ph_q = deque()
            ph_q2 = deque()
            PIPE2 = int(os.environ.get("EVS8_PIPE2", str(2 * PIPE + 4)))

            def flush(keep):
                while len(ph_q) > keep:
                    ph_q.popleft()()

            def flush2(keep):
                while len(ph_q2) > keep:
                    ph_q2.popleft()()

            def get_psum(sqi):
                if sqi not in psum_tiles:
                    psum_tiles[sqi] = psump.tile([P, 2 * W], F32, tag="ps",
                                                 name=f"ps{sqi % int(os.environ.get('EVS8_PSB', '2'))}",
                                                 uniquify=True)
                return psum_tiles[sqi]

            def emit_mms(gtiles, mg):
                for j, (c, sqi, gh, r, first, last) in enumerate(gtiles):
                    ch, lo = divmod(c, CHUNK)
                    rhs_t = get_chunk(ch)
                    ps = get_psum(sqi)
                    nc.tensor.matmul(
                        ps[WY * gh:WY * (gh + 1),
                           2 * WX * r:2 * WX * (r + 1)],
                        lhsT=mg[:, j * WY:(j + 1) * WY],
                        rhs=rhs_t[:, lo * 2 * WX:(lo + 1) * 2 * WX],
                        start=first, stop=last,
                        tile_position=(0, WY * gh))

            def make_abs(ci, gw, dg, cell):
                def phA():
                    ae = abs_seq[ci]
                    zg = zp.tile([P, G * WY], F16, tag="zg")
                    if ae == "act":
                        nc.scalar.activation(zg[:, 0:gw], dg[:, 0:gw],
                                             Act.Abs)
                    elif ae == "dve":
                        nc.vector.scalar_tensor_tensor(
                            zg[:, 0:gw], dg[:, 0:gw], -1.0, dg[:, 0:gw],
                            op0=Alu.mult, op1=Alu.max)
                    else:
                        nc.gpsimd.scalar_tensor_tensor(
                            zg[:, 0:gw], dg[:, 0:gw], -1.0, dg[:, 0:gw],
                            op0=Alu.mult, op1=Alu.max)
                    cell["zg"] = zg

                return phA

            def make_mms(route, gtiles, gw, cell):
                def phB():
                    if route in ("scat", "hostm"):
                        mg = cell["mg"]
                    else:
                        zg = cell["zg"]
                        mg = mp.tile([P, G * WY], F16, tag="mg")
                        nc.vector.tensor_scalar(mg[:, 0:gw], zg[:, 0:gw],
                                                1.0, 0.0,
                                                op0=Alu.subtract,
                                                op1=Alu.min)
                    emit_mms(gtiles, mg)

                return phB

            def vslot(plane, qi):
                base = ((plane % 3) * NQ + qi) * W
                return vring[:, base:base + W]

            def make_drain_pieces(sqi):
                """Merge the two bin half-planes into the V ring (copy +
                add, same column count as two copies) and DMA finalized
                planes; halves output bytes vs shipping both halves."""
                si, qi = divmod(sqi, NQ)
                rows = min(P, H - P * qi)

                def pc_act():
                    ps = psum_tiles[sqi]
                    pv = ps[0:rows, :].rearrange("p (x b) -> p x b", b=2)
                    # open plane si+1 with segment si's b1 half
                    nc.scalar.activation(vslot(si + 1, qi)[0:rows, :],
                                         pv[:, :, 1], Act.Copy)

                def pc_dve():
                    ps = psum_tiles.pop(sqi)
                    pv = ps[0:rows, :].rearrange("p (x b) -> p x b", b=2)
                    vv = vslot(si, qi)[0:rows, :]
                    if si == 0:
                        nc.vector.tensor_copy(vv, pv[:, :, 0])
                    else:
                        nc.vector.scalar_tensor_tensor(
                            vv, pv[:, :, 0], 0.0, vv,
                            op0=Alu.add, op1=Alu.add)
                    # plane si's q-block is final now
                    dma_rr(out=out_d[si, qi, 0:rows, :], in_=vv)
                    if si == NSEG - 1:
                        dma_rr(out=out_d[NSEG, qi, 0:rows, :],
                               in_=vslot(NSEG, qi)[0:rows, :])

                return [pc_act, pc_dve]

            last_to_sq = {v: k for k, v in sq_last_tile.items()}
            pending_pieces = deque()
            ci = 0  # chain-group ordinal
            for gi, (g0, g1) in enumerate(groups):
                gtiles = tiles[g0:g1]
                gn = len(gtiles)
                c0 = gtiles[0][0]
                # touch psums in stream order so pool cycling stays sane
                for tl in gtiles:
                    get_psum(tl[1])
                cell = {}
                if route_seq[gi] == "hostm":
                    # lhsT shipped prebuilt from the host: one DMA, no
                    # engine work at all
                    hb = hostm_base[gi]
                    mg = mhp.tile([P, G * WY], F8, tag="mh", name="mh")
                    dma_rr(out=mg[:, 0:gn * WY],
                           in_=mh_d[:, WY * hb:WY * (hb + gn)])
                    cell["mg"] = mg
                    ph_q.append(lambda: None)
                elif route_seq[gi] == "scat":
                    # whole m tile built by one gpsimd scatter of the
                    # per-event (1-fy, fy) taps
                    sb = scat_base[gi]
                    mg = mp.tile([P, G * WY], F16, tag="mgs", name="mgs")
                    nc.gpsimd.local_scatter(
                        mg[:, 0:gn * WY], ydt[:, 2 * sb:2 * (sb + gn)],
                        yit[:, 2 * sb:2 * (sb + gn)], channels=P,
                        num_elems=gn * WY, num_idxs=2 * gn)
                    cell["mg"] = mg
                    ph_q.append(lambda: None)
                else:
                    dg = dp.tile([P, G * WY], F16, tag="dgt")
                    iob = io16[:].rearrange("p (o c) -> p o c", o=1) \
                        .to_broadcast([P, gn, WY])
                    cb = chain_base[gi]
                    yb = yt[:, cb:cb + gn] \
                        .rearrange("p (g o) -> p g o", o=1) \
                        .to_broadcast([P, gn, WY])
                    dgv = dg[:, 0:gn * WY].rearrange("p (g c) -> p g c",
                                                     g=gn)
                    if sub_seq[ci] == "dve":
                        nc.vector.tensor_tensor(dgv, iob, yb,
                                                op=Alu.subtract)
                    else:
                        nc.gpsimd.tensor_tensor(dgv, iob, yb,
                                                op=Alu.subtract)
                    ph_q.append(make_abs(ci, gn * WY, dg, cell))
                    ci += 1
                ph_q2.append(make_mms(route_seq[gi], gtiles, gn * WY, cell))
                # drains queue behind the completing group's matmuls
                for ti in range(g0, g0 + gn):
                    if ti in last_to_sq:
                        for pc in make_drain_pieces(last_to_sq[ti]):
                            ph_q2.append(pc)
                flush(PIPE)
                flush2(PIPE2)
            flush(0)
            flush2(0)
    nc.finalize()
    return nc


def kernel(events, lengths):
    events = np.ascontiguousarray(events, dtype=np.float32)
    lengths = np.asarray(lengths)
    B = int(lengths.shape[0])
    offs = np.zeros(B + 1, np.int64)
    offs[1:] = np.cumsum(lengths)

    packs = []
    counts = np.zeros((B, NKEY), np.int64)
    for bi in range(B):
        c, pk = _host_prep(events[offs[bi]:offs[bi + 1]])
        counts[bi] = c
        packs.append(pk)

    tiles_per_key = np.maximum(1, -(-counts.max(axis=0) // P)).astype(np.int64)
    T_tot = int(tiles_per_key.sum())

    key = (tuple(tiles_per_key.tolist()), T_tot, G, CHUNK, SCAT, TAPER,
           HOSTM, SUB_DVE, ABS_ACT, ABS_DVE, DRAIN_CA, PIPE,
           os.environ.get("EVS8_GSMALL", ""),
           os.environ.get("EVS8_PIPE2", ""),
           os.environ.get("EVS8_PSB", "2"),
           os.environ.get("EVS8_DB", "8"), os.environ.get("EVS8_MB", "8"))
    if key not in _prog_cache:
        _prog_cache[key] = _build_program(tiles_per_key, T_tot)
    nc = _prog_cache[key]

    in_maps = [_pack_core(pk, tiles_per_key, T_tot) for pk in packs]
    trace = bool(int(os.environ.get("EVS_TRACE", "0")))
    res = run_bass_kernel_spmd(nc, in_maps, core_ids=list(range(B)),
                               trace=trace)
    global last_results
    last_results = res

    out = np.zeros((B, BINS, H, W), np.float32)
    for bi in range(B):
        ov = np.asarray(res.results[bi]["outv"]).astype(np.float32)
        out[bi] = ov.reshape(BINS, NQ * P, W)[:, :H] * (1.0 / 8.0)
    return out


last_results = None


if __name__ == "__main__":
    # tiny smoke test with synthetic events
    rng = np.random.default_rng(0)
    B0, NP0 = 8, 2000
    N0 = B0 * NP0
    x = rng.uniform(0, W - 1, N0).astype(np.float32)
    y = rng.uniform(0, H - 1, N0).astype(np.float32)
    t = np.sort(rng.uniform(0, 1, (B0, NP0)).astype(np.float32), axis=1).ravel()
    p = (2.0 * rng.integers(0, 2, N0) - 1).astype(np.float32)
    b = np.repeat(np.arange(B0), NP0).astype(np.float32)
    ev = np.stack([x, y, t, p, b], axis=1)
    ln = np.full(B0, NP0, np.int32)
    out = kernel(ev, ln)
    # numpy reference
    ref = np.zeros((B0, BINS, H, W), np.float64)
    for bi in range(B0):
        sl = slice(bi * NP0, (bi + 1) * NP0)
        xx, yy, tt2, pp = x[sl], y[sl], t[sl], p[sl]
        t0, tN = tt2[0], tt2[-1]
        ts = (BINS - 1) * np.clip((tt2 - t0) / (tN - t0), 0, 1)
        import itertools
        for xr_f, yr_f, br_f in itertools.product([np.floor, np.ceil], repeat=3):
            xr, yr, br = xr_f(xx), yr_f(yy), br_f(ts)
            valid = (((xr != xx) | (xr_f is np.floor))
                     & ((yr != yy) | (yr_f is np.floor))
                     & ((br != ts) | (br_f is np.floor))
                     & (xr < W) & (yr < H) & (br < BINS))
            kb = lambda a_: np.maximum(0, 1 - np.abs(a_))
            val = np.where(valid, pp * kb(xr - xx) * kb(yr - yy) * kb(br - ts), 0)
            np.add.at(ref[bi].ravel(),
                      np.where(valid, (xr + yr * W + br * H * W).astype(np.int64), 0),
                      val)
    num = np.linalg.norm((out - ref).ravel())
    den = np.linalg.norm(ref.ravel())
    print("smoke rel l2 err:", num / max(den, 1e-30))
    print("smoke max abs err:", np.abs(out - ref).max())


# revision 52
# speedup vs baseline: 1.0511x; 1.0170x over previous
"""EventVolumeSurface trilinear voxel-grid kernel for Trainium2 (Bass/Tile).

v10 strategy (data-parallel over batch, 1 batch -> 1 NeuronCore):
  Events are bucketed by (time-segment s in [0,9), y-window q32 = iy>>5 in
  [0,15), x-window r32 = ix>>5 in [0,20)) with straddle duplication at the
  32-boundaries.  Tiles of 128 event slots stream in bucket order, batched
  into groups of G=63.

  Host ships, per 128-event tile slot:
    - rhs [128, T*64] fp8 e3m4: 64 interleaved columns (2*cx + b) holding
      sign * 8 * p * w_b * hx[cx]; hx = 2-tap x hat, w_0 = 1-f, w_1 = f
      (time-bin weights); x8 keeps values in e3m4's normal range (host
      divides by 8 at unshard); sign matches the group route below.
    - scat groups (75%): per-slot y taps (1-fy, fy) f16 + int16
      group-local scatter indices (-1 = skip, handles window straddle).
    - hostm groups (28%): the +hat lhsT tile prebuilt on the host, in
      fp8 e3m4 (unscaled hat values, so no psum-scale mixing).
    - chain groups (0% by default, kept as a tuning fallback): yhat f32.

  Device m-tile build (columns cost engine time; partitions are free):
    scat:  ONE gpsimd local_scatter builds the whole [128, 63*32] +hat
           lhsT group on Pool (~47ns/tile, replaces sub/abs/min chains)
    hostm: one DMA, zero engine work (spends spare DMA bandwidth to
           relieve Pool; the 75/25 split equalizes Pool vs DMA)
    chain: DVE broadcast-subtract -> ACT Abs -> DVE fused min (= -hat)
    MM: psum[32*g:+32, 64*r:+64] += m_j^T @ rhs_j  (f16 x fp8e3, 64 out
        cols ~36ns; tile_position puts the 32 out rows at partition 32*g)
  PSUM is one [128, 1280] f32 tile per (s, q128) stripe with interleaved
  (x, bin) columns (bin-strided matmul out APs are broken on HW).  Drains
  merge the two bin half-planes on-device into a 3-deep V ring (ACT copy
  of the b1 half opens plane s+1, DVE add of the b0 half finalizes plane
  s), so the output DMA ships each plane once: 6.1 MB f16 total.  A
  two-stage deferral (ABS at lag PIPE, CLAMP+matmuls+drains at lag PIPE2)
  keeps cross-engine producers off in-order queue heads, and DMA issue
  alternates the SP/ACT queues.

  TimelineSim: 106.9us/core (v7 baseline: 308.7us).  Rel L2 err 1.44e-2,
  dominated by the deterministic fp8-e3m4 rhs/lhsT quantization.
"""

import os
import sys
from collections import deque

import numpy as np

sys.path.insert(0, "/opt/trn_rl_repo")

import ml_dtypes

import concourse.bass as bass
import concourse.bacc as bacc
import concourse.mybir as mybir
import concourse.tile as tile
from concourse.bass_utils import run_bass_kernel_spmd

H, W, BINS = 480, 640, 10
NSEG = BINS - 1          # 9 time segments (t*=9 folds into seg 8 with f=1)
P = 128
WY = 32                  # y-window width
WX = 32                  # x-window width
NQ32 = (H + WY - 1) // WY   # 15
NR32 = (W + WX - 1) // WX   # 20
NQ = 4                   # 128-tall psum stripes
NKEY = NSEG * NQ32 * NR32   # 2700 buckets
N_CORES = 8
G = int(os.environ.get("EVS8_G", "63"))        # tiles per batched group
CHUNK = int(os.environ.get("EVS8_CHUNK", "63"))  # rhs tiles per DMA chunk

# static engine mixes (percent)
SCAT = int(os.environ.get("EVS8_SCAT", "71"))   # share of groups built by
#                                         gpsimd local_scatter
HOSTM = int(os.environ.get("EVS8_HOSTM", "29"))  # share with host-shipped
#                                         m tiles (zero engine work; rest
#                                         of the groups use the chain)
SUB_DVE = int(os.environ.get("EVS8_SUB_DVE", "100"))     # rest -> Pool
ABS_ACT = int(os.environ.get("EVS8_ABS_ACT", "100"))
ABS_DVE = int(os.environ.get("EVS8_ABS_DVE", "0"))      # rest -> Pool
DRAIN_CA = int(os.environ.get("EVS8_DRAIN_CA", "704"))  # ACT cols of 1280
PIPE = int(os.environ.get("EVS8_PIPE", "3"))
CHB = int(os.environ.get("EVS8_CHB", "6"))
TAPER = int(os.environ.get("EVS8_TAPER", "126"))

F32 = mybir.dt.float32
F16 = mybir.dt.float16
F8 = mybir.dt.float8e3

_prog_cache: dict = {}


def _host_prep(ev):
    """Bucket one batch's events; returns (counts[NKEY], pack data)."""
    if ev.shape[0] == 0:
        ev = np.array([[0.0, 0.0, 0.25, 0.0, 0.0],
                       [0.0, 0.0, 0.75, 0.0, 0.0]], np.float32)
    x = ev[:, 0].astype(np.float64)
    y = ev[:, 1].astype(np.float64)
    t = ev[:, 2].astype(np.float64)
    p = ev[:, 3].astype(np.float32)
    t0 = t[0]
    tN = t[-1]
    denom = tN - t0
    if denom > 0:
        tp = (BINS - 1) * np.clip((t - t0) / denom, 0.0, 1.0)
    else:
        tp = np.zeros_like(t)
    s = np.clip(np.floor(tp).astype(np.int32), 0, NSEG - 1)
    f = (tp - s).astype(np.float64)

    iy = np.floor(y).astype(np.int64)
    icy = np.ceil(y).astype(np.int64)
    ix = np.floor(x).astype(np.int64)
    icx = np.ceil(x).astype(np.int64)
    qf, qc = iy // WY, icy // WY
    rf, rc = ix // WX, icx // WX
    n = len(x)
    idx0 = np.arange(n, dtype=np.int64)

    ys = qf != qc
    xs = rf != rc
    both = ys & xs
    inst_idx = np.concatenate([idx0, idx0[ys], idx0[xs], idx0[both]])
    inst_q = np.concatenate([qf, qc[ys], qf[xs], qc[both]])
    inst_r = np.concatenate([rf, rf[ys], rc[xs], rc[both]])
    key = (s[inst_idx] * NQ32 + inst_q) * NR32 + inst_r
    counts = np.bincount(key, minlength=NKEY)
    return counts, (x, y, f, p, ix, inst_idx, inst_q, inst_r, key)


def _pack_core(pack, tiles_per_key, T_tot):
    x, y, f, p, ix, inst_idx, inst_q, inst_r, key = pack
    col0 = np.zeros(NKEY + 1, np.int64)
    col0[1:] = np.cumsum(tiles_per_key)
    order = np.argsort(key, kind="stable")
    skey = key[order]
    sidx = inst_idx[order]
    sq = inst_q[order]
    sr = inst_r[order]
    group_start = np.searchsorted(skey, np.arange(NKEY))
    rank = np.arange(len(skey)) - group_start[skey]
    slot = col0[skey] * P + rank
    part = (slot % P).astype(np.int64)
    col = (slot // P).astype(np.int64)

    yrel = (y[sidx] - WY * sq).astype(np.float64)

    # schedule-derived per-tile info: group base + rhs sign (+hat for scat
    # groups, -hat for chain groups)
    (groups, route_seq, _, _, chain_base, scat_base,
     hostm_base) = _schedule(T_tot)
    g0s = np.array([g[0] for g in groups], np.int64)
    tile_g = np.searchsorted(g0s, np.arange(T_tot), side="right") - 1
    rcode = np.array([{"scat": 0, "hostm": 1, "chain": 2}[route_seq[g]]
                      for g in range(len(groups))])
    tile_sign = np.where(rcode[tile_g] == 2, -1.0, 1.0)   # [T_tot]
    tile_base = g0s[tile_g]            # group start tile of each tile
    cbase = np.array(chain_base, np.int64)
    sbase = np.array(scat_base, np.int64)
    hbase = np.array(hostm_base, np.int64)
    # compact per-tile destination columns
    last_n = groups[-1][1] - groups[-1][0]
    lr = rcode[-1]
    n_scat = max(1, int(sbase[-1] + (last_n if lr == 0 else 0)))
    n_hostm = max(1, int(hbase[-1] + (last_n if lr == 1 else 0)))
    n_chain = max(1, int(cbase[-1] + (last_n if lr == 2 else 0)))
    tile_local = np.arange(T_tot) - tile_base
    tile_ccol = cbase[tile_g] + tile_local      # chain-compact column
    tile_scol = sbase[tile_g] + tile_local      # scat-compact column
    tile_hcol = hbase[tile_g] + tile_local      # hostm-compact column

    Yc = np.zeros((P, n_chain), np.float32)
    chain_slots = rcode[tile_g[col]] == 2
    Yc[part[chain_slots], tile_ccol[col[chain_slots]]] = \
        yrel[chain_slots].astype(np.float32)

    # y taps; consumed by the scat groups (as scatter data/idx) and the
    # hostm groups (as a dense prebuilt +hat lhsT)
    cy0 = np.floor(yrel).astype(np.int64)
    fy = yrel - cy0
    YD = np.zeros((P, 2 * n_scat), np.float16)
    YI = np.full((P, 2 * n_scat), -1, np.int16)
    MH = np.zeros((P, WY * n_hostm), ml_dtypes.float8_e3m4)
    local = (col - tile_base[col]) * WY
    scol2 = 2 * tile_scol[col]
    ss = rcode[tile_g[col]] == 0
    hh = rcode[tile_g[col]] == 1
    hcol = tile_hcol[col] * WY
    for k, (c, h) in enumerate(((cy0, 1.0 - fy), (cy0 + 1, fy))):
        ok = ss & (c >= 0) & (c < WY)
        YD[part[ss], scol2[ss] + k] = h[ss].astype(np.float16)
        YI[part[ok], scol2[ok] + k] = (local[ok] + c[ok]).astype(np.int16)
        okh = hh & (c >= 0) & (c < WY)
        MH[part[okh], hcol[okh] + c[okh]] = \
            h[okh].astype(ml_dtypes.float8_e3m4)

    # rhs: 64 interleaved cols (2*cx + b) = sign * 8 * p * w_b * hx[cx]
    xi = x[sidx]
    fi = f[sidx]
    pi = p[sidx].astype(np.float64)
    ixi = ix[sidx]
    s8 = 8.0 * tile_sign[col] * pi
    cf = ixi - WX * sr           # floor-tap col (may be -1 for ceil dups)
    cc = cf + 1                  # ceil-tap col (may be 32 for floor side)
    hx1 = xi - ixi               # ceil-tap weight
    hx0 = 1.0 - hx1
    w0 = s8 * (1.0 - fi)
    w1 = s8 * fi
    RHS = np.zeros((P, T_tot, 2 * WX), np.float32)
    for valid, c, h in ((cf >= 0, cf, hx0), (cc < WX, cc, hx1)):
        for b, wv in ((0, w0), (1, w1)):
            RHS[part[valid], col[valid], 2 * c[valid] + b] = \
                (h * wv)[valid].astype(np.float32)
    RHS8 = RHS.reshape(P, T_tot * 2 * WX).astype(ml_dtypes.float8_e3m4)
    return {"ev_y": Yc, "ev_rhs": RHS8, "ev_yd": YD,
            "ev_yi": YI, "ev_mh": MH}


def _mix_seq(fracs, n):
    """Maximally-even interleave (error diffusion) of engine choices."""
    tot = max(1, sum(fracs.values()))
    fr = {k: v / tot for k, v in fracs.items()}
    cnt = {k: 0 for k in fr}
    seq = []
    for i in range(n):
        pick = max(fr, key=lambda k: fr[k] * (i + 1) - cnt[k])
        cnt[pick] += 1
        seq.append(pick)
    return seq


def _schedule(T_tot):
    """Group slices + per-group route assignment, shared host/device.

    Returns (groups, route_seq, sub_seq, abs_seq, chain_base, scat_base):
    the *_base lists give each group's start offset in the compact
    chain-y / scat-data arrays (in tiles)."""
    bounds = list(range(0, max(0, T_tot - TAPER), G))
    g_small = int(os.environ.get("EVS8_GSMALL", "10"))
    bounds += list(range(max(0, T_tot - TAPER), T_tot, g_small))
    bounds.append(T_tot)
    groups = [(a, b) for a, b in zip(bounds[:-1], bounds[1:]) if b > a]
    n = len(groups)
    route_seq = _mix_seq({"scat": SCAT, "hostm": HOSTM,
                          "chain": max(0, 100 - SCAT - HOSTM)}, n)
    nchain = sum(1 for r in route_seq if r == "chain")
    sub_seq = _mix_seq({"dve": SUB_DVE, "pool": 100 - SUB_DVE}, nchain)
    abs_seq = _mix_seq({"act": ABS_ACT, "dve": ABS_DVE,
                        "pool": max(0, 100 - ABS_ACT - ABS_DVE)}, nchain)
    chain_base, scat_base, hostm_base = [], [], []
    cb = sb = hb = 0
    for g, (a, b) in enumerate(groups):
        chain_base.append(cb)
        scat_base.append(sb)
        hostm_base.append(hb)
        if route_seq[g] == "scat":
            sb += b - a
        elif route_seq[g] == "hostm":
            hb += b - a
        else:
            cb += b - a
    return (groups, route_seq, sub_seq, abs_seq, chain_base, scat_base,
            hostm_base)


def _build_program(tiles_per_key, T_tot):
    nc = bacc.Bacc("TRN2", debug=False)
    (groups, route_seq, sub_seq, abs_seq, chain_base, scat_base,
     hostm_base) = _schedule(T_tot)
    ngroups = len(groups)
    last_n = groups[-1][1] - groups[-1][0]
    lr = route_seq[-1]
    n_scat = max(1, scat_base[-1] + (last_n if lr == "scat" else 0))
    n_hostm = max(1, hostm_base[-1] + (last_n if lr == "hostm" else 0))
    n_chain = max(1, chain_base[-1] + (last_n if lr == "chain" else 0))
    y_d = nc.dram_tensor("ev_y", [P, n_chain], F32, kind="ExternalInput")
    yd_d = nc.dram_tensor("ev_yd", [P, 2 * n_scat], F16,
                          kind="ExternalInput")
    yi_d = nc.dram_tensor("ev_yi", [P, 2 * n_scat], mybir.dt.int16,
                          kind="ExternalInput")
    rhs_d = nc.dram_tensor("ev_rhs", [P, T_tot * 2 * WX], F8,
                           kind="ExternalInput")
    mh_d = nc.dram_tensor("ev_mh", [P, WY * n_hostm], F8,
                          kind="ExternalInput")
    out_d = nc.dram_tensor("outv", [BINS, NQ, P, W], F16,
                           kind="ExternalOutput")

    col0 = np.zeros(NKEY + 1, np.int64)
    col0[1:] = np.cumsum(tiles_per_key)

    Alu = mybir.AluOpType
    Act = mybir.ActivationFunctionType

    # global tile stream: (c, sq_idx, ghat, r, first, last)
    tiles = []
    sq_last_tile = {}
    for si in range(NSEG):
        for qi in range(NQ):
            sqi = si * NQ + qi
            for q32 in range(4 * qi, min(4 * qi + 4, NQ32)):
                for r in range(NR32):
                    k = (si * NQ32 + q32) * NR32 + r
                    ntile = int(tiles_per_key[k])
                    cbase = int(col0[k])
                    for j in range(ntile):
                        tiles.append((cbase + j, sqi, q32 - 4 * qi, r,
                                      j == 0, j == ntile - 1))
                        sq_last_tile[sqi] = len(tiles) - 1
    assert len(tiles) == T_tot


    with tile.TileContext(nc) as tc:
        with (
            tc.tile_pool(name="persist", bufs=1) as persist,
            tc.tile_pool(name="psum", bufs=int(os.environ.get("EVS8_PSB", "2")), space="PSUM") as psump,
            tc.tile_pool(name="chunk", bufs=CHB) as chp,
            tc.tile_pool(name="dg", bufs=int(os.environ.get("EVS8_DB", "8"))) as dp,
            tc.tile_pool(name="zg", bufs=int(os.environ.get("EVS8_DB", "8"))) as zp,
            tc.tile_pool(name="mg", bufs=int(os.environ.get("EVS8_MB", "8"))) as mp,
            tc.tile_pool(name="mh", bufs=4) as mhp,
        ):
            dmaq = [nc.sync, nc.scalar]
            dmaqi = [0]

            def dma_rr(**kw):
                eng = dmaq[dmaqi[0] % len(dmaq)]
                dmaqi[0] += 1
                eng.dma_start(**kw)

            # --- y values (small first chunk so group 0 starts ASAP)
            yt = persist.tile([P, n_chain], F32, tag="yt")
            ydt = persist.tile([P, 2 * n_scat], F16, tag="ydt")
            yit = persist.tile([P, 2 * n_scat], mybir.dt.int16, tag="yit")
            for tot, dst, src_d, m in ((n_chain, yt, y_d, 1),
                                       (n_scat, ydt, yd_d, 2),
                                       (n_scat, yit, yi_d, 2)):
                cuts = [0, tot // 16, tot // 4, tot // 2,
                        3 * tot // 4, tot]
                for y0, y1 in zip(cuts[:-1], cuts[1:]):
                    if y1 > y0:
                        dma_rr(out=dst[:, m * y0:m * y1],
                               in_=src_d[:, m * y0:m * y1])

            # --- V ring: 3 generations x NQ q-blocks of one output plane
            VR = 3
            vring = persist.tile([P, VR * NQ * W], F16, tag="vring")

            # --- constants: io = iota cols 0..31 (f16), ACT table warm
            ioi = persist.tile([P, WY], mybir.dt.int32, tag="ioi")
            nc.gpsimd.iota(ioi[:], pattern=[[1, WY]], base=0,
                           channel_multiplier=0)
            io16 = persist.tile([P, WY], F16, tag="io16")
            nc.vector.tensor_copy(io16[:], ioi[:])
            warm = persist.tile([1, 1], F16, tag="warm")
            nc.vector.memset(warm[:], 0.0)
            nc.scalar.activation(warm[:], warm[:], Act.Abs)

            # --- rhs chunk streaming
            chunk_tiles: dict = {}

            def get_chunk(ch):
                if ch not in chunk_tiles:
                    t = chp.tile([P, CHUNK * 2 * WX], F8, tag="ch")
                    lo = ch * CHUNK * 2 * WX
                    hi = min((ch + 1) * CHUNK * 2 * WX, T_tot * 2 * WX)
                    dma_rr(out=t[:, 0:hi - lo], in_=rhs_d[:, lo:hi])
                    chunk_tiles[ch] = t
                return chunk_tiles[ch]

            for _pc in range(4):
                get_chunk(_pc)

            psum_tiles: dict = {}
            ph_q = deque()
            ph_q2 = deque()
            PIPE2 = int(os.environ.get("EVS8_PIPE2", str(2 * PIPE + 4)))

            def flush(keep):
                while len(ph_q) > keep:
                    ph_q.popleft()()

            def flush2(keep):
                while len(ph_q2) > keep:
                    ph_q2.popleft()()

            def get_psum(sqi):
                if sqi not in psum_tiles:
                    psum_tiles[sqi] = psump.tile([P, 2 * W], F32, tag="ps",
                                                 name=f"ps{sqi % int(os.environ.get('EVS8_PSB', '2'))}",
                                                 uniquify=True)
                return psum_tiles[sqi]

            def emit_mms(gtiles, mg):
                for j, (c, sqi, gh, r, first, last) in enumerate(gtiles):
                    ch, lo = divmod(c, CHUNK)
                    rhs_t = get_chunk(ch)
                    ps = get_psum(sqi)
                    nc.tensor.matmul(
                        ps[WY * gh:WY * (gh + 1),
                           2 * WX * r:2 * WX * (r + 1)],
                        lhsT=mg[:, j * WY:(j + 1) * WY],
                        rhs=rhs_t[:, lo * 2 * WX:(lo + 1) * 2 * WX],
                        start=first, stop=last,
                        tile_position=(0, WY * gh))

            def make_abs(ci, gw, dg, cell):
                def phA():
                    ae = abs_seq[ci]
                    zg = zp.tile([P, G * WY], F16, tag="zg")
                    if ae == "act":
                        nc.scalar.activation(zg[:, 0:gw], dg[:, 0:gw],
                                             Act.Abs)
                    elif ae == "dve":
                        nc.vector.scalar_tensor_tensor(
                            zg[:, 0:gw], dg[:, 0:gw], -1.0, dg[:, 0:gw],
                            op0=Alu.mult, op1=Alu.max)
                    else:
                        nc.gpsimd.scalar_tensor_tensor(
                            zg[:, 0:gw], dg[:, 0:gw], -1.0, dg[:, 0:gw],
                            op0=Alu.mult, op1=Alu.max)
                    cell["zg"] = zg

                return phA

            def make_mms(route, gtiles, gw, cell):
                def phB():
                    if route in ("scat", "hostm"):
                        mg = cell["mg"]
                    else:
                        zg = cell["zg"]
                        mg = mp.tile([P, G * WY], F16, tag="mg")
                        nc.vector.tensor_scalar(mg[:, 0:gw], zg[:, 0:gw],
                                                1.0, 0.0,
                                                op0=Alu.subtract,
                                                op1=Alu.min)
                    emit_mms(gtiles, mg)

                return phB

            def vslot(plane, qi):
                base = ((plane % 3) * NQ + qi) * W
                return vring[:, base:base + W]

            def make_drain_pieces(sqi):
                """Merge the two bin half-planes into the V ring (copy +
                add, same column count as two copies) and DMA finalized
                planes; halves output bytes vs shipping both halves."""
                si, qi = divmod(sqi, NQ)
                rows = min(P, H - P * qi)

                def pc_act():
                    ps = psum_tiles[sqi]
                    pv = ps[0:rows, :].rearrange("p (x b) -> p x b", b=2)
                    # open plane si+1 with segment si's b1 half
                    nc.scalar.activation(vslot(si + 1, qi)[0:rows, :],
                                         pv[:, :, 1], Act.Copy)

                def pc_dve():
                    ps = psum_tiles.pop(sqi)
                    pv = ps[0:rows, :].rearrange("p (x b) -> p x b", b=2)
                    vv = vslot(si, qi)[0:rows, :]
                    if si == 0:
                        nc.vector.tensor_copy(vv, pv[:, :, 0])
                    else:
                        nc.vector.scalar_tensor_tensor(
                            vv, pv[:, :, 0], 0.0, vv,
                            op0=Alu.add, op1=Alu.add)
                    # plane si's q-block is final now
                    dma_rr(out=out_d[si, qi, 0:rows, :], in_=vv)
                    if si == NSEG - 1:
                        dma_rr(out=out_d[NSEG, qi, 0:rows, :],
                               in_=vslot(NSEG, qi)[0:rows, :])

                return [pc_act, pc_dve]

            last_to_sq = {v: k for k, v in sq_last_tile.items()}
            pending_pieces = deque()
            ci = 0  # chain-group ordinal
            for gi, (g0, g1) in enumerate(groups):
                gtiles = tiles[g0:g1]
                gn = len(gtiles)
                c0 = gtiles[0][0]
                # touch psums in stream order so pool cycling stays sane
                for tl in gtiles:
                    get_psum(tl[1])
                cell = {}
                if route_seq[gi] == "hostm":
                    # lhsT shipped prebuilt from the host: one DMA, no
                    # engine work at all
                    hb = hostm_base[gi]
                    mg = mhp.tile([P, G * WY], F8, tag="mh", name="mh")
                    dma_rr(out=mg[:, 0:gn * WY],
                           in_=mh_d[:, WY * hb:WY * (hb + gn)])
                    cell["mg"] = mg
                    ph_q.append(lambda: None)
                elif route_seq[gi] == "scat":
                    # whole m tile built by one gpsimd scatter of the
                    # per-event (1-fy, fy) taps
                    sb = scat_base[gi]
                    mg = mp.tile([P, G * WY], F16, tag="mgs", name="mgs")
                    nc.gpsimd.local_scatter(
                        mg[:, 0:gn * WY], ydt[:, 2 * sb:2 * (sb + gn)],
                        yit[:, 2 * sb:2 * (sb + gn)], channels=P,
                        num_elems=gn * WY, num_idxs=2 * gn)
                    cell["mg"] = mg
                    ph_q.append(lambda: None)
                else:
                    dg = dp.tile([P, G * WY], F16, tag="dgt")
                    iob = io16[:].rearrange("p (o c) -> p o c", o=1) \
                        .to_broadcast([P, gn, WY])
                    cb = chain_base[gi]
                    yb = yt[:, cb:cb + gn] \
                        .rearrange("p (g o) -> p g o", o=1) \
                        .to_broadcast([P, gn, WY])
                    dgv = dg[:, 0:gn * WY].rearrange("p (g c) -> p g c",
                                                     g=gn)
                    if sub_seq[ci] == "dve":
                        nc.vector.tensor_tensor(dgv, iob, yb,
                                                op=Alu.subtract)
                    else:
                        nc.gpsimd.tensor_tensor(dgv, iob, yb,
                                                op=Alu.subtract)
                    ph_q.append(make_abs(ci, gn * WY, dg, cell))
                    ci += 1
                ph_q2.append(make_mms(route_seq[gi], gtiles, gn * WY, cell))
                # drains queue behind the completing group's matmuls
                for ti in range(g0, g0 + gn):
                    if ti in last_to_sq:
                        for pc in make_drain_pieces(last_to_sq[ti]):
                            ph_q2.append(pc)
                flush(PIPE)
                flush2(PIPE2)
            flush(0)
            flush2(0)
    nc.finalize()
    return nc


def kernel(events, lengths):
    events = np.ascontiguousarray(events, dtype=np.float32)
    lengths = np.asarray(lengths)
    B = int(lengths.shape[0])
    offs = np.zeros(B + 1, np.int64)
    offs[1:] = np.cumsum(lengths)

    packs = []
    counts = np.zeros((B, NKEY), np.int64)
    for bi in range(B):
        c, pk = _host_prep(events[offs[bi]:offs[bi + 1]])
        counts[bi] = c
        packs.append(pk)

    tiles_per_key = np.maximum(1, -(-counts.max(axis=0) // P)).astype(np.int64)
    T_tot = int(tiles_per_key.sum())

    key = (tuple(tiles_per_key.tolist()), T_tot, G, CHUNK, SCAT, TAPER,
           HOSTM, SUB_DVE, ABS_ACT, ABS_DVE, DRAIN_CA, PIPE,
           os.environ.get("EVS8_GSMALL", ""),
           os.environ.get("EVS8_PIPE2", ""),
           os.environ.get("EVS8_PSB", "2"),
           os.environ.get("EVS8_DB", "8"), os.environ.get("EVS8_MB", "8"))
    if key not in _prog_cache:
        _prog_cache[key] = _build_program(tiles_per_key, T_tot)
    nc = _prog_cache[key]

    in_maps = [_pack_core(pk, tiles_per_key, T_tot) for pk in packs]
    trace = bool(int(os.environ.get("EVS_TRACE", "0")))
    res = run_bass_kernel_spmd(nc, in_maps, core_ids=list(range(B)),
                               trace=trace)
    global last_results
    last_results = res

    out = np.zeros((B, BINS, H, W), np.float32)
    for bi in range(B):
        ov = np.asarray(res.results[bi]["outv"]).astype(np.float32)
        out[bi] = ov.reshape(BINS, NQ * P, W)[:, :H] * (1.0 / 8.0)
    return out


last_results = None


if __name__ == "__main__":
    # tiny smoke test with synthetic events
    rng = np.random.default_rng(0)
    B0, NP0 = 8, 2000
    N0 = B0 * NP0
    x = rng.uniform(0, W - 1, N0).astype(np.float32)
    y = rng.uniform(0, H - 1, N0).astype(np.float32)
    t = np.sort(rng.uniform(0, 1, (B0, NP0)).astype(np.float32), axis=1).ravel()
    p = (2.0 * rng.integers(0, 2, N0) - 1).astype(np.float32)
    b = np.repeat(np.arange(B0), NP0).astype(np.float32)
    ev = np.stack([x, y, t, p, b], axis=1)
    ln = np.full(B0, NP0, np.int32)
    out = kernel(ev, ln)
    # numpy reference
    ref = np.zeros((B0, BINS, H, W), np.float64)
    for bi in range(B0):
        sl = slice(bi * NP0, (bi + 1) * NP0)
        xx, yy, tt2, pp = x[sl], y[sl], t[sl], p[sl]
        t0, tN = tt2[0], tt2[-1]
        ts = (BINS - 1) * np.clip((tt2 - t0) / (tN - t0), 0, 1)
        import itertools
        for xr_f, yr_f, br_f in itertools.product([np.floor, np.ceil], repeat=3):
            xr, yr, br = xr_f(xx), yr_f(yy), br_f(ts)
            valid = (((xr != xx) | (xr_f is np.floor))
                     & ((yr != yy) | (yr_f is np.floor))
                     & ((br != ts) | (br_f is np.floor))
                     & (xr < W) & (yr < H) & (br < BINS))
            kb = lambda a_: np.maximum(0, 1 - np.abs(a_))
            val = np.where(valid, pp * kb(xr - xx) * kb(yr - yy) * kb(br - ts), 0)
            np.add.at(ref[bi].ravel(),
                      np.where(valid, (xr + yr * W + br * H * W).astype(np.int64), 0),
                      val)
    num = np.linalg.norm((out - ref).ravel())
    den = np.linalg.norm(ref.ravel())
    print("smoke rel l2 err:", num / max(den, 1e-30))
    print("smoke max abs err:", np.abs(out - ref).max())
